# revision 7
# baseline (speedup 1.0000x reference)
"""ChildSum TreeLSTM on TRN2, 8-core SPMD Bass/Tile kernel — v3.

v3 vs v2 (178861 ns):
- L4-split sharding: each core owns 32 of the 256 level-4 subtrees
  (global subtree j = 8*s + core), so real leaves are balanced across
  cores: leaf block shrinks 8192 -> 5632 columns and the 5461 childless
  level-7 nodes run the cheap leaf recipe instead of the full gate path.
- fp8(e4m3) DoubleRow matmuls for the bulk: 2 contraction rows per
  partition at 0.5 cycles/output-column -> 4x the f32r rate.  x packed
  [128, 2, C] + [23, 2, C] (301 rows + zero pad = 128*2 + 23*2).
- fi folded into the forget-gate PSUM group via a broadcast (stride-0)
  rhs matmul -> the [P,4L] DVE add and fi copy disappear.
- child h_sum folded into the i/u PSUM group via 4 stride-4 child
  matmuls -> the DVE 4-way reduce disappears.
- elementwise in bf16 on DVE (2x mode), h states stored fp8 for the
  next level's DoubleRow matmuls, c states bf16.
- one AllGather of the 256 level-4 (h,c) states; levels 3..0 (85 nodes)
  replicated on every core in bf16.
"""

import numpy as np

D = 300
H = 256
P = 128
KB = 4
N_CORES = 8
N_NODES = 65536

ST = 32          # subtrees per core
SL = 22          # leaf-carrying subtree slots per core
C8 = SL * 256    # 5632 leaf columns
C7 = ST * 64     # 2048
I7 = SL * 64     # 1408 internal-recipe level-7 columns
T7 = C7 - I7     # 640 leaf-recipe level-7 columns
C6 = ST * 16     # 512
C5 = ST * 4      # 128
C4 = ST          # 32
CTOP = 85        # nodes 0..84 replicated
X8, X7, X6, X5, X4, XT = 0, 5632, 7680, 8192, 8320, 8352
CX = XT + CTOP   # 8437 x columns per core
S8, S7, S6, S5, S4 = 21845, 5461, 1365, 341, 85


def _q8(a):
    import ml_dtypes
    return np.asarray(a, dtype=ml_dtypes.float8_e4m3)


def _qb(a):
    import ml_dtypes
    return np.asarray(a, dtype=ml_dtypes.bfloat16)


def prep_inputs(inputs, ix_w, ix_b, ih_w, ih_b, ux_w, ux_b, uh_w, uh_b,
                fi_w, fi_b, fh_w, fh_b):
    n = inputs.shape[0]
    assert n == N_NODES

    # v: u-preactivation of a padded column is ~0 -> h=c~0
    v = np.linalg.lstsq(ux_w.astype(np.float64),
                        -(ux_b + uh_b).astype(np.float64), rcond=None)[0]
    vcol = np.concatenate([v.astype(np.float32), [1.0]])    # row 300 = 1

    xT = np.concatenate([inputs.T.astype(np.float32),
                         np.ones((1, n), np.float32)], axis=0)  # [301, N]

    # --- weights, shared across cores ---
    Wp = np.zeros((302, 2 * H), np.float32)
    Wp[:D, :H] = ix_w.T
    Wp[:D, H:] = ux_w.T
    Wp[D, :H] = ix_b + ih_b
    Wp[D, H:] = ux_b + uh_b
    Wh = np.concatenate([ih_w.T, uh_w.T], axis=1)           # [256, 512]
    Wfi = np.zeros((302, H), np.float32)
    Wfi[:D] = fi_w.T
    Wfi[D] = fi_b + fh_b
    Wfh = np.ascontiguousarray(fh_w.T)                      # [256, 256]

    def pack_a(W, blocks):         # rows 0..255 -> [128, 128*len(blocks)*?]
        cols = []
        for c0 in blocks:
            for i in range(2):
                cols.append(W[128 * i:128 * (i + 1), c0:c0 + 128])
        return np.concatenate(cols, axis=1)

    def pack_b(W, blocks):         # rows 256..301 (+pad) -> [23, ...]
        cols = []
        for c0 in blocks:
            for i in range(2):
                cols.append(W[256 + 23 * i:256 + 23 * (i + 1), c0:c0 + 128])
        return np.concatenate(cols, axis=1)

    mt4 = [0, 128, 256, 384]
    mt2 = [0, 128]
    weights = {
        "wpa": _q8(pack_a(Wp, mt4)), "wpb": _q8(pack_b(Wp, mt4)),
        "wha": _q8(pack_a(Wh, mt4)),
        "wfha": _q8(pack_a(Wfh, mt2)),
        "wfia": _q8(pack_a(Wfi, mt2)), "wfib": _q8(pack_b(Wfi, mt2)),
        "wp_bf": _qb(Wp[:301]), "wh_bf": _qb(Wh),
        "wfi_bf": _qb(Wfi[:301]), "wfh_bf": _qb(Wfh),
        "xtop": _qb(xT[:, :CTOP]),
    }

    in_maps = []
    for g in range(N_CORES):
        xc = np.empty((302, CX), np.float32)
        xc[301] = 0.0
        js = 8 * np.arange(ST) + g
        # leaf block
        for s in range(SL):
            n0 = S8 + 256 * js[s]
            cnt = int(np.clip(n - n0, 0, 256))
            if cnt > 0:
                xc[:301, X8 + 256 * s:X8 + 256 * s + cnt] = xT[:, n0:n0 + cnt]
            if cnt < 256:
                xc[:301, X8 + 256 * s + cnt:X8 + 256 * (s + 1)] = \
                    vcol[:, None]
        # level blocks 7..4 (all real nodes)
        for (base, off, w) in ((S7, X7, 64), (S6, X6, 16),
                               (S5, X5, 4), (S4, X4, 1)):
            idx = (base + w * js[:, None] + np.arange(w)[None, :]).ravel()
            xc[:301, off:off + ST * w] = xT[:, idx]
        xc[:301, XT:] = xT[:, :CTOP]
        xa = _q8(np.concatenate([xc[0:128], xc[128:256]], axis=1))
        xb = _q8(np.concatenate([xc[256:279], xc[279:302]], axis=1))
        m = dict(weights)
        m["xa"] = xa
        m["xb"] = xb
        in_maps.append(m)
    return in_maps


def build_program(n=N_NODES, debug=False, timing=False):
    import concourse.bass as bass
    import concourse.tile as tile
    from concourse import bacc, mybir

    f32 = mybir.dt.float32
    bf16 = mybir.dt.bfloat16
    fp8 = mybir.dt.float8e4
    AF = mybir.ActivationFunctionType
    AX = mybir.AxisListType
    PM = mybir.MatmulPerfMode
    DR = PM.DoubleRow

    nc = bacc.Bacc("TRN2", target_bir_lowering=False, debug=debug,
                   num_devices=N_CORES)

    xa_d = nc.dram_tensor("xa", [P, 2 * CX], fp8, kind="ExternalInput")
    xb_d = nc.dram_tensor("xb", [23, 2 * CX], fp8, kind="ExternalInput")
    xtop_d = nc.dram_tensor("xtop", [301, CTOP], bf16, kind="ExternalInput")
    wpa_d = nc.dram_tensor("wpa", [P, 1024], fp8, kind="ExternalInput")
    wpb_d = nc.dram_tensor("wpb", [23, 1024], fp8, kind="ExternalInput")
    wha_d = nc.dram_tensor("wha", [P, 1024], fp8, kind="ExternalInput")
    wfha_d = nc.dram_tensor("wfha", [P, 512], fp8, kind="ExternalInput")
    wfia_d = nc.dram_tensor("wfia", [P, 512], fp8, kind="ExternalInput")
    wfib_d = nc.dram_tensor("wfib", [23, 512], fp8, kind="ExternalInput")
    wp_bf_d = nc.dram_tensor("wp_bf", [301, 512], bf16, kind="ExternalInput")
    wh_bf_d = nc.dram_tensor("wh_bf", [256, 512], bf16, kind="ExternalInput")
    wfi_bf_d = nc.dram_tensor("wfi_bf", [301, 256], bf16,
                              kind="ExternalInput")
    wfh_bf_d = nc.dram_tensor("wfh_bf", [256, 256], bf16,
                              kind="ExternalInput")
    h0_d = nc.dram_tensor("h0", [P, 2], f32, kind="ExternalOutput")
    c0_d = nc.dram_tensor("c0", [P, 2], f32, kind="ExternalOutput")

    with tile.TileContext(nc) as tc:
        import contextlib
        with contextlib.ExitStack() as stack:
            wpool = stack.enter_context(tc.tile_pool(name="w", bufs=1))
            state = stack.enter_context(tc.tile_pool(name="state", bufs=1))
            work = stack.enter_context(tc.tile_pool(name="work", bufs=2))
            psum = stack.enter_context(
                tc.tile_pool(name="psum", bufs=1, space="PSUM"))
            dram = stack.enter_context(
                tc.tile_pool(name="dram", bufs=1, space="DRAM"))

            # ---------------- weights ----------------
            wpa = wpool.tile([P, 1024], fp8, name="wpa")
            wpb = wpool.tile([23, 1024], fp8, name="wpb")
            wha = wpool.tile([P, 1024], fp8, name="wha")
            wfha = wpool.tile([P, 512], fp8, name="wfha")
            wfia = wpool.tile([P, 512], fp8, name="wfia")
            wfib = wpool.tile([23, 512], fp8, name="wfib")
            nc.gpsimd.dma_start(wpa[:], wpa_d[:, :])
            nc.gpsimd.dma_start(wpb[:], wpb_d[:, :])
            nc.gpsimd.dma_start(wha[:], wha_d[:, :])
            nc.scalar.dma_start(wfha[:], wfha_d[:, :])
            nc.scalar.dma_start(wfia[:], wfia_d[:, :])
            nc.scalar.dma_start(wfib[:], wfib_d[:, :])
            # bf16 top weights, contraction chunks (0:128),(128:256),(256:301)
            wp_bf = [wpool.tile([128, 512], bf16, name="wp_bf0"),
                     wpool.tile([128, 512], bf16, name="wp_bf1"),
                     wpool.tile([45, 512], bf16, name="wp_bf2")]
            wfi_bf = [wpool.tile([128, 256], bf16, name="wfi_bf0"),
                      wpool.tile([128, 256], bf16, name="wfi_bf1"),
                      wpool.tile([45, 256], bf16, name="wfi_bf2")]
            wh_bf = [wpool.tile([128, 512], bf16, name="wh_bf0"),
                     wpool.tile([128, 512], bf16, name="wh_bf1")]
            wfh_bf = [wpool.tile([128, 256], bf16, name="wfh_bf0"),
                      wpool.tile([128, 256], bf16, name="wfh_bf1")]
            for k, (r0, r1) in enumerate(((0, 128), (128, 256), (256, 301))):
                nc.gpsimd.dma_start(wp_bf[k][:], wp_bf_d[r0:r1, :])
                nc.gpsimd.dma_start(wfi_bf[k][:], wfi_bf_d[r0:r1, :])
            for k in range(2):
                nc.gpsimd.dma_start(wh_bf[k][:], wh_bf_d[128 * k:128 * (k + 1), :])
                nc.gpsimd.dma_start(wfh_bf[k][:], wfh_bf_d[128 * k:128 * (k + 1), :])
            xt = [wpool.tile([128, CTOP], bf16, name="xt0"),
                  wpool.tile([128, CTOP], bf16, name="xt1"),
                  wpool.tile([45, CTOP], bf16, name="xt2")]
            for k, (r0, r1) in enumerate(((0, 128), (128, 256), (256, 301))):
                nc.scalar.dma_start(xt[k][:], xtop_d[r0:r1, :])

            # ---------------- x ----------------
            xa = state.tile([P, 2 * CX], fp8, name="xa")
            xb = state.tile([23, 2 * CX], fp8, name="xb")
            xav = xa[:].rearrange("k (two c) -> k two c", two=2)
            xbv = xb[:].rearrange("k (two c) -> k two c", two=2)
            xa_dv = xa_d[:, :].rearrange("k (two c) -> k two c", two=2)
            xb_dv = xb_d[:, :].rearrange("k (two c) -> k two c", two=2)
            for (c0, c1) in ((0, 2816), (2816, 5632), (5632, CX)):
                nc.sync.dma_start(xav[:, :, c0:c1], xa_dv[:, :, c0:c1])
            nc.scalar.dma_start(xbv[:, :, :], xb_dv[:, :, :])

            # ---------------- states ----------------
            h8 = state.tile([P, 2 * C8], fp8, name="h8")
            c8 = state.tile([P, 2 * C8], bf16, name="c8")
            h7 = state.tile([P, 2 * C7], fp8, name="h7")
            c7 = state.tile([P, 2 * C7], bf16, name="c7")
            h6 = state.tile([P, 2 * C6], fp8, name="h6")
            c6 = state.tile([P, 2 * C6], bf16, name="c6")
            h5 = state.tile([P, 2 * C5], fp8, name="h5")
            c5 = state.tile([P, 2 * C5], bf16, name="c5")
            h4f = state.tile([P, 2 * C4], f32, name="h4f")
            c4f = state.tile([P, 2 * C4], f32, name="c4f")
            h4gf = state.tile([P, 2 * 256], f32, name="h4gf")
            h4g = state.tile([P, 2 * 256], bf16, name="h4g")
            c4g = state.tile([P, 2 * 256], f32, name="c4g")
            h3 = state.tile([P, 2 * 64], bf16, name="h3")
            c3 = state.tile([P, 2 * 64], f32, name="c3")
            h2 = state.tile([P, 2 * 16], bf16, name="h2")
            c2 = state.tile([P, 2 * 16], f32, name="c2")
            h1 = state.tile([P, 2 * 4], bf16, name="h1")
            c1 = state.tile([P, 2 * 4], f32, name="c1")
            h0t = state.tile([P, 2], f32, name="h0t")
            c0t = state.tile([P, 2], f32, name="c0t")
            # per-level fc accumulators (bf16 except L4/top in f32)
            fc7 = state.tile([P, 2 * I7], bf16, name="fc7")
            fc6 = state.tile([P, 2 * C6], bf16, name="fc6")
            fc5 = state.tile([P, 2 * C5], bf16, name="fc5")
            fc4 = state.tile([P, 2 * C4], f32, name="fc4")
            fct = state.tile([P, 2 * 64], f32, name="fct")

            def sv(t, cols):       # state view [128, 2, cols-slice]
                return t[:].rearrange("k (two c) -> k two c", two=2)

            def wv_a(t, blk):      # fp8 DR lhsT view, A part
                return (t[:, 256 * blk:256 * (blk + 1)]
                        .rearrange("k (two m) -> k two m", two=2))

            def wv_b(t, blk):
                return (t[:, 256 * blk:256 * (blk + 1)]
                        .rearrange("k (two m) -> k two m", two=2))

            # ============ fp8 emitters ============

            def iu_chunk(xc0, L, tag, child=None, cc0=0):
                """i/u pre-acts for L parent cols at x cols [xc0, xc0+L).
                child=(h_tile, Ctot) adds the 4-child h sum (stride-4).
                Returns (pi, pu) psum tiles [P, 2*512] (cols 0:L, 512:512+L).
                """
                pi = psum.tile([P, 1024], f32, name=f"pi{tag}", tag="pi")
                pu = psum.tile([P, 1024], f32, name=f"pu{tag}", tag="pu")
                for gate, pt in ((0, pi), (1, pu)):
                    for mt in range(2):
                        out = pt[:, 512 * mt:512 * mt + L]
                        blk = 2 * gate + mt
                        mms = [(wv_a(wpa, blk), xav[:, :, xc0:xc0 + L], DR),
                               (wv_b(wpb, blk), xbv[:, :, xc0:xc0 + L], DR)]
                        if child is not None:
                            ht, Ct = child
                            hv = ht[:].rearrange("k (two c) -> k two c",
                                                 two=2)
                            for k in range(4):
                                mms.append(
                                    (wv_a(wha, blk),
                                     hv[:, :, cc0 + k:cc0 + 4 * L:4], DR))
                        for q, (w, r, pm) in enumerate(mms):
                            nc.tensor.matmul(out, w, r, start=(q == 0),
                                             stop=(q == len(mms) - 1),
                                             perf_mode=pm)
                return pi, pu

            def forget_chunk(lq0, Lf, xc0, tag, child_h, child_c, Cc, cc0,
                             fc_t, Mfc):
                """Forget path for Lf parents (<=256), x col xc0, children at
                child cols [cc0, cc0+4Lf). Writes fc_t cols [lq0, lq0+Lf) per
                mt (stride Mfc)."""
                hv = child_h[:].rearrange("k (two c) -> k two c", two=2)
                for mt in range(2):
                    pf = psum.tile([P, 1024], f32, name=f"pf{tag}_{mt}",
                                   tag="pf", bufs=2)
                    xva = (xav[:, :, xc0:xc0 + Lf].unsqueeze(3)
                           .broadcast_to([P, 2, Lf, 4]))
                    xvb = (xbv[:, :, xc0:xc0 + Lf].unsqueeze(3)
                           .broadcast_to([23, 2, Lf, 4]))
                    nc.tensor.matmul(pf[:, 0:4 * Lf], wv_a(wfha, mt),
                                     hv[:, :, cc0:cc0 + 4 * Lf],
                                     start=True, stop=False, perf_mode=DR)
                    nc.tensor.matmul(pf[:, 0:4 * Lf], wv_a(wfia, mt), xva,
                                     start=False, stop=False, perf_mode=DR)
                    nc.tensor.matmul(pf[:, 0:4 * Lf], wv_b(wfib, mt), xvb,
                                     start=False, stop=True, perf_mode=DR)
                    ft = work.tile([P, 1024], bf16, name=f"f{tag}_{mt}",
                                   tag="fM")
                    nc.scalar.activation(ft[:, 0:4 * Lf], pf[:, 0:4 * Lf],
                                         AF.Sigmoid)
                    fcc = work.tile([P, 1024], bf16, name=f"fx{tag}_{mt}",
                                    tag="fccM")
                    nc.vector.tensor_mul(
                        fcc[:, 0:4 * Lf], ft[:, 0:4 * Lf],
                        child_c[:, Cc * mt + cc0:Cc * mt + cc0 + 4 * Lf])
                    with nc.allow_low_precision(reason="fc bf16"):
                        nc.vector.reduce_sum(
                            fc_t[:, Mfc * mt + lq0:Mfc * mt + lq0 + Lf],
                            fcc[:, 0:4 * Lf]
                            .rearrange("k (l four) -> k l four", four=4),
                            axis=AX.X)

            def cio_chunk(pi, pu, L, tag, out_h, out_c, Cout, oc0,
                          fc_t=None, Mfc=None, lq0=0, h_dt_f32=False,
                          defer_h=False):
                """activations + c for L cols from iu psums; h = tanh(c)."""
                it = work.tile([P, 1024], bf16, name=f"i{tag}", tag="it")
                ut = work.tile([P, 1024], bf16, name=f"u{tag}", tag="ut")
                piv = pi[:].rearrange("k (mt c) -> k mt c", mt=2)[:, :, 0:L]
                puv = pu[:].rearrange("k (mt c) -> k mt c", mt=2)[:, :, 0:L]
                itv = it[:].rearrange("k (mt c) -> k mt c", mt=2)[:, :, 0:L]
                utv = ut[:].rearrange("k (mt c) -> k mt c", mt=2)[:, :, 0:L]
                nc.scalar.activation(itv, piv, AF.Sigmoid)
                nc.scalar.activation(utv, puv, AF.Tanh)
                ocv = sv(out_c, 0)[:, :, oc0:oc0 + L]
                if fc_t is None:
                    nc.vector.tensor_mul(ocv, itv, utv)
                else:
                    tt = work.tile([P, 1024], bf16, name=f"t{tag}", tag="tt")
                    ttv = (tt[:].rearrange("k (mt c) -> k mt c", mt=2)
                           [:, :, 0:L])
                    nc.vector.tensor_mul(ttv, itv, utv)
                    fcv = (fc_t[:].rearrange("k (mt c) -> k mt c", mt=2)
                           [:, :, lq0:lq0 + L])
                    nc.vector.tensor_add(ocv, ttv, fcv)
                if not defer_h:
                    emit_h(out_h, out_c, oc0, L, h_dt_f32)

            def emit_h(out_h, out_c, oc0, L, h_dt_f32=False):
                nc.scalar.activation(
                    sv(out_h, 0)[:, :, oc0:oc0 + L],
                    sv(out_c, 0)[:, :, oc0:oc0 + L], AF.Tanh)

            # ============ bf16 (top) emitters ============

            def iu_chunk_bf(xc0, L, tag, child=None, cc0=0, Cc=0):
                pi = psum.tile([P, 1024], f32, name=f"pi{tag}", tag="pi")
                pu = psum.tile([P, 1024], f32, name=f"pu{tag}", tag="pu")
                for gate, pt in ((0, pi), (1, pu)):
                    for mt in range(2):
                        out = pt[:, 512 * mt:512 * mt + L]
                        w0 = 256 * gate + 128 * mt
                        mms = [(wp_bf[k][:, w0:w0 + 128],
                                xt[k][:, xc0:xc0 + L]) for k in range(3)]
                        if child is not None:
                            for half in range(2):
                                for k in range(4):
                                    mms.append(
                                        (wh_bf[half][:, w0:w0 + 128],
                                         child[:, Cc * half + cc0 + k:
                                               Cc * half + cc0 + 4 * L:4]))
                        for q, (w, r) in enumerate(mms):
                            nc.tensor.matmul(out, w, r, start=(q == 0),
                                             stop=(q == len(mms) - 1))
                return pi, pu

            def forget_chunk_bf(lq0, Lf, xc0, tag, child_h, child_c, Cc,
                                cc0, fc_t, Mfc):
                for mt in range(2):
                    pf = psum.tile([P, 1024], f32, name=f"pf{tag}_{mt}",
                                   tag="pf", bufs=2)
                    w0 = 128 * mt
                    for half in range(2):
                        nc.tensor.matmul(
                            pf[:, 0:4 * Lf], wfh_bf[half][:, w0:w0 + 128],
                            child_h[:, Cc * half + cc0:
                                    Cc * half + cc0 + 4 * Lf],
                            start=(half == 0), stop=False)
                    for k in range(3):
                        kp = 128 if k < 2 else 45
                        xvk = (xt[k][:, xc0:xc0 + Lf].unsqueeze(2)
                               .broadcast_to([kp, Lf, 4]))
                        nc.tensor.matmul(
                            pf[:, 0:4 * Lf], wfi_bf[k][:, w0:w0 + 128],
                            xvk, start=False, stop=(k == 2))
                    ft = work.tile([P, 1024], bf16, name=f"f{tag}_{mt}",
                                   tag="fM")
                    nc.scalar.activation(ft[:, 0:4 * Lf], pf[:, 0:4 * Lf],
                                         AF.Sigmoid)
                    fcc = work.tile([P, 1024], f32, name=f"fx{tag}_{mt}",
                                    tag="fccT")
                    nc.vector.tensor_mul(
                        fcc[:, 0:4 * Lf], ft[:, 0:4 * Lf],
                        child_c[:, Cc * mt + cc0:Cc * mt + cc0 + 4 * Lf])
                    nc.vector.reduce_sum(
                        fc_t[:, Mfc * mt + lq0:Mfc * mt + lq0 + Lf],
                        fcc[:, 0:4 * Lf]
                        .rearrange("k (l four) -> k l four", four=4),
                        axis=AX.X)

            def cio_chunk_top(pi, pu, L, tag, out_h, out_c, oc0, fc_t, lq0,
                              Mfc):
                it = work.tile([P, 1024], bf16, name=f"i{tag}", tag="it")
                ut = work.tile([P, 1024], bf16, name=f"u{tag}", tag="ut")
                piv = pi[:].rearrange("k (mt c) -> k mt c", mt=2)[:, :, 0:L]
                puv = pu[:].rearrange("k (mt c) -> k mt c", mt=2)[:, :, 0:L]
                itv = it[:].rearrange("k (mt c) -> k mt c", mt=2)[:, :, 0:L]
                utv = ut[:].rearrange("k (mt c) -> k mt c", mt=2)[:, :, 0:L]
                nc.scalar.activation(itv, piv, AF.Sigmoid)
                nc.scalar.activation(utv, puv, AF.Tanh)
                tt = work.tile([P, 1024], f32, name=f"t{tag}", tag="tt32")
                ttv = tt[:].rearrange("k (mt c) -> k mt c", mt=2)[:, :, 0:L]
                nc.vector.tensor_mul(ttv, itv, utv)
                ocv = sv(out_c, 0)[:, :, oc0:oc0 + L]
                fcv = (fc_t[:].rearrange("k (mt c) -> k mt c", mt=2)
                       [:, :, lq0:lq0 + L])
                nc.vector.tensor_add(ocv, ttv, fcv)
                nc.scalar.activation(sv(out_h, 0)[:, :, oc0:oc0 + L],
                                     ocv, AF.Tanh)

            # ============ emission schedule ============

            def leaf_chunk(j):     # 512 leaf cols
                c0 = 512 * j
                L = min(512, C8 - c0)
                pi, pu = iu_chunk(X8 + c0, L, f"l{j}")
                cio_chunk(pi, pu, L, f"l{j}", h8, c8, C8, c0, defer_h=True)

            def leaf_h(j):         # tanh over 1024 cols
                c0 = 1024 * j
                L = min(1024, C8 - c0)
                emit_h(h8, c8, c0, L)

            def tail_chunk(j):     # leaf-recipe level-7 cols
                c0 = I7 + 512 * j
                L = min(512, C7 - c0)
                pi, pu = iu_chunk(X7 + c0, L, f"t{j}")
                cio_chunk(pi, pu, L, f"t{j}", h7, c7, C7, c0)

            LVL = {
                7: dict(h=h7, c=c7, C=C7, fc=fc7, M=I7, xo=X7,
                        ch=h8, cc=c8, Cc=C8, npar=I7),
                6: dict(h=h6, c=c6, C=C6, fc=fc6, M=C6, xo=X6,
                        ch=h7, cc=c7, Cc=C7, npar=C6),
                5: dict(h=h5, c=c5, C=C5, fc=fc5, M=C5, xo=X5,
                        ch=h6, cc=c6, Cc=C6, npar=C5),
                4: dict(h=h4f, c=c4f, C=C4, fc=fc4, M=C4, xo=X4,
                        ch=h5, cc=c5, Cc=C5, npar=C4),
            }

            def fchunk(l, q):      # forget chunk q (256 parents) of level l
                v = LVL[l]
                q0 = 256 * q
                Lf = min(256, v["npar"] - q0)
                forget_chunk(q0, Lf, v["xo"] + q0, f"L{l}q{q}", v["ch"],
                             v["cc"], v["Cc"], 4 * q0, v["fc"], v["M"])

            def ichunk(l, j):      # iu+c+h chunk j (512 parents) of level l
                v = LVL[l]
                c0 = 512 * j
                L = min(512, v["npar"] - c0)
                pi, pu = iu_chunk(v["xo"] + c0, L, f"L{l}i{j}",
                                  child=(v["ch"], v["Cc"]), cc0=4 * c0)
                cio_chunk(pi, pu, L, f"L{l}i{j}", v["h"], v["c"], v["C"],
                          c0, fc_t=v["fc"], Mfc=v["M"], lq0=c0,
                          h_dt_f32=(l == 4))

            # --- interleaved schedule ---
            # leaf chunks: 11; tail: 2; L7: forget 6, iu 3; L6: f 2, iu 1;
            # L5: f 1, iu 1 (128); L4: f 1, iu 1 (32)
            # deps: leaf_h(q) <- leaf chunks 2q,2q+1;  fchunk(7,q) <- leaf_h(q)
            # ichunk(7,j) <- fchunk(7,2j),(7,2j+1) + leaf_h(2j),(2j+1)
            sched = [
                lambda: leaf_chunk(0), lambda: tail_chunk(0),
                lambda: leaf_chunk(1), lambda: tail_chunk(1),
                lambda: leaf_h(0),
                lambda: leaf_chunk(2), lambda: fchunk(7, 0),
                lambda: leaf_chunk(3), lambda: leaf_h(1),
                lambda: leaf_chunk(4), lambda: fchunk(7, 1),
                lambda: leaf_chunk(5), lambda: leaf_h(2),
                lambda: ichunk(7, 0),
                lambda: leaf_chunk(6), lambda: fchunk(7, 2),
                lambda: leaf_chunk(7), lambda: leaf_h(3),
                lambda: leaf_chunk(8), lambda: fchunk(7, 3),
                lambda: leaf_chunk(9), lambda: leaf_h(4),
                lambda: leaf_chunk(10), lambda: fchunk(7, 4),
                lambda: ichunk(7, 1), lambda: leaf_h(5),
                lambda: fchunk(7, 5), lambda: ichunk(7, 2),
                lambda: fchunk(6, 0), lambda: fchunk(6, 1),
                lambda: ichunk(6, 0),
                lambda: fchunk(5, 0), lambda: ichunk(5, 0),
                lambda: fchunk(4, 0), lambda: ichunk(4, 0),
            ]
            for fn in sched:
                fn()

            # ---- AllGather the 256 level-4 states ----
            ag_in = dram.tile([1, 16384], f32, name="ag_in")
            ag_out = dram.tile([N_CORES, 16384], f32, name="ag_out")
            nc.gpsimd.dma_start(
                ag_in[:, 0:8192].rearrange("o (p c) -> (o p) c", p=P),
                h4f[:])
            nc.gpsimd.dma_start(
                ag_in[:, 8192:16384].rearrange("o (p c) -> (o p) c", p=P),
                c4f[:])
            if timing:
                for g in range(N_CORES):
                    nc.sync.dma_start(ag_out[g:g + 1, :], ag_in[:])
            else:
                from concourse import mybir as _mb
                nc.gpsimd.collective_compute(
                    "AllGather", _mb.AluOpType.bypass,
                    replica_groups=[list(range(N_CORES))],
                    ins=[ag_in[:].opt()], outs=[ag_out[:].opt()])
            h4gv = h4gf[:].rearrange("k (two j) -> k two j", two=2)
            c4gv = c4g[:].rearrange("k (two j) -> k two j", two=2)
            for g in range(N_CORES):
                src_h = (ag_out[g:g + 1, 0:8192]
                         .rearrange("o (p two s) -> (o p) two s", p=P, two=2))
                src_c = (ag_out[g:g + 1, 8192:16384]
                         .rearrange("o (p two s) -> (o p) two s", p=P, two=2))
                nc.gpsimd.dma_start(h4gv[:, :, g:256:8], src_h)
                nc.gpsimd.dma_start(c4gv[:, :, g:256:8], src_c)
            nc.vector.tensor_copy(h4g[:], h4gf[:])

            # ---- replicated top: L3 (64), L2 (16), L1 (4), L0 (1) ----
            TOP = [
                (64, 21, h4g, c4g, 256, h3, c3, 64),
                (16, 5, h3, c3, 64, h2, c2, 16),
                (4, 1, h2, c2, 16, h1, c1, 4),
                (1, 0, h1, c1, 4, h0t, c0t, 1),
            ]
            for li, (L, x0, ch, cc, Cc, oh, oc, Co) in enumerate(TOP):
                forget_chunk_bf(0, L, x0, f"T{li}", ch, cc, Cc, 0, fct, 64)
                pi, pu = iu_chunk_bf(x0, L, f"T{li}", child=ch, Cc=Cc)
                cio_chunk_top(pi, pu, L, f"T{li}", oh, oc, 0, fct, 0, 64)

            nc.sync.dma_start(h0_d[:, :], h0t[:])
            nc.sync.dma_start(c0_d[:, :], c0t[:])

    nc.compile()
    return nc


# ---------------------------------------------------------------------------
# self-contained entry point: kernel(**inputs) -> (h[0], c[0])
# ---------------------------------------------------------------------------
_CACHE = {}


def _ensure_paths():
    import sys
    for p in ("/opt/trn_rl_repo",):
        if p not in sys.path:
            sys.path.insert(0, p)


def _get_runner():
    if "runner" in _CACHE:
        return _CACHE["runner"]
    _ensure_paths()
    import jax
    from jax.sharding import Mesh, PartitionSpec, NamedSharding
    from jax.experimental.shard_map import shard_map
    from concourse import bass2jax, mybir

    nc = build_program(N_NODES)
    bass2jax.install_neuronx_cc_hook()
    partition_name = (nc.partition_id_tensor.name
                      if nc.partition_id_tensor else None)
    in_names, out_names, out_avals, zero_outs = [], [], [], []
    for alloc in nc.m.functions[0].allocations:
        if not isinstance(alloc, mybir.MemoryLocationSet):
            continue
        name = alloc.memorylocations[0].name
        if alloc.kind == "ExternalInput":
            if name != partition_name:
                in_names.append(name)
        elif alloc.kind == "ExternalOutput":
            out_names.append(name)
            shape = tuple(alloc.tensor_shape)
            dtype = mybir.dt.np(alloc.dtype)
            out_avals.append(jax.core.ShapedArray(shape, dtype))
            zero_outs.append(np.zeros(shape, dtype))
    all_in = list(in_names) + list(out_names)
    if partition_name is not None:
        all_in.append(partition_name)

    def _body(*args):
        operands = list(args)
        if partition_name is not None:
            operands.append(bass2jax.partition_id_tensor())
        return tuple(bass2jax._bass_exec_p.bind(
            *operands, out_avals=tuple(out_avals), in_names=tuple(all_in),
            out_names=tuple(out_names), lowering_input_output_aliases=(),
            sim_require_finite=True, sim_require_nnan=True, nc=nc))

    devices = jax.devices()[:N_CORES]
    assert len(devices) == N_CORES, (
        f"need {N_CORES} neuron devices, found {len(jax.devices())}")
    mesh = Mesh(np.asarray(devices), ("core",))
    nio = len(in_names) + len(out_names)
    sharded = jax.jit(
        shard_map(_body, mesh=mesh,
                  in_specs=(PartitionSpec("core"),) * nio,
                  out_specs=(PartitionSpec("core"),) * len(out_names),
                  check_rep=False),
        keep_unused=True)
    sh = NamedSharding(mesh, PartitionSpec("core"))
    runner = dict(run=sharded, in_names=in_names, out_names=out_names,
                  zero_outs=zero_outs, sh=sh, jax=jax)
    _CACHE["runner"] = runner
    return runner


def kernel(inputs, ix_w, ix_b, ih_w, ih_b, ux_w, ux_b, uh_w, uh_b,
           fi_w, fi_b, fh_w, fh_b):
    """ChildSum TreeLSTM over a complete 4-ary tree of 65536 nodes on 8
    NeuronCores (SPMD, one AllGather at the level-4 frontier).
    Returns (h[0], c[0]) as float32 arrays of shape (256,)."""
    assert np.asarray(inputs).shape == (N_NODES, D)
    in_maps = prep_inputs(
        np.asarray(inputs, np.float32),
        np.asarray(ix_w, np.float32), np.asarray(ix_b, np.float32),
        np.asarray(ih_w, np.float32), np.asarray(ih_b, np.float32),
        np.asarray(ux_w, np.float32), np.asarray(ux_b, np.float32),
        np.asarray(uh_w, np.float32), np.asarray(uh_b, np.float32),
        np.asarray(fi_w, np.float32), np.asarray(fi_b, np.float32),
        np.asarray(fh_w, np.float32), np.asarray(fh_b, np.float32))
    r = _get_runner()
    jax = r["jax"]
    concat = [np.concatenate([in_maps[c][nm] for c in range(N_CORES)], axis=0)
              for nm in r["in_names"]]
    dev_in = [jax.device_put(a, r["sh"]) for a in concat]
    dev_zero = [jax.device_put(
        np.zeros((N_CORES * z.shape[0], *z.shape[1:]), z.dtype), r["sh"])
        for z in r["zero_outs"]]
    outs = r["run"](*dev_in, *dev_zero)
    res = {nm: np.asarray(outs[i]).reshape(N_CORES, P, 2)[0]
           for i, nm in enumerate(r["out_names"])}
    h0 = res["h0"].T.reshape(2 * P).astype(np.float32)
    c0 = res["c0"].T.reshape(2 * P).astype(np.float32)
    return h0, c0


# revision 9
# speedup vs baseline: 1.3544x; 1.3544x over previous
"""ChildSum TreeLSTM on TRN2, 8-core SPMD Bass/Tile kernel — v3.

v3 vs v2 (178861 ns):
- L4-split sharding: each core owns 32 of the 256 level-4 subtrees
  (global subtree j = 8*s + core), so real leaves are balanced across
  cores: leaf block shrinks 8192 -> 5632 columns and the 5461 childless
  level-7 nodes run the cheap leaf recipe instead of the full gate path.
- fp8(e4m3) DoubleRow matmuls for the bulk: 2 contraction rows per
  partition at 0.5 cycles/output-column -> 4x the f32r rate.  x packed
  [128, 2, C] + [23, 2, C] (301 rows + zero pad = 128*2 + 23*2).
- fi folded into the forget-gate PSUM group via a broadcast (stride-0)
  rhs matmul -> the [P,4L] DVE add and fi copy disappear.
- child h_sum folded into the i/u PSUM group via 4 stride-4 child
  matmuls -> the DVE 4-way reduce disappears.
- elementwise in bf16 on DVE (2x mode), h states stored fp8 for the
  next level's DoubleRow matmuls, c states bf16.
- one AllGather of the 256 level-4 (h,c) states; levels 3..0 (85 nodes)
  replicated on every core in bf16.
"""

import numpy as np

D = 300
H = 256
P = 128
KB = 4
N_CORES = 8
N_NODES = 65536

ST = 32          # subtrees per core
SL = 22          # leaf-carrying subtree slots per core
C8 = SL * 256    # 5632 leaf columns
C7 = ST * 64     # 2048
I7 = SL * 64     # 1408 internal-recipe level-7 columns
T7 = C7 - I7     # 640 leaf-recipe level-7 columns
C6 = ST * 16     # 512
C5 = ST * 4      # 128
C4 = ST          # 32
CTOP = 85        # nodes 0..84 replicated
X8, X7, X6, X5, X4, XT = 0, 5632, 7680, 8192, 8320, 8352
CX = XT + CTOP   # 8437 x columns per core
S8, S7, S6, S5, S4 = 21845, 5461, 1365, 341, 85


def _q8(a):
    import ml_dtypes
    return np.asarray(a, dtype=ml_dtypes.float8_e4m3)


def _qb(a):
    import ml_dtypes
    return np.asarray(a, dtype=ml_dtypes.bfloat16)


def prep_inputs(inputs, ix_w, ix_b, ih_w, ih_b, ux_w, ux_b, uh_w, uh_b,
                fi_w, fi_b, fh_w, fh_b):
    n = inputs.shape[0]
    assert n == N_NODES

    # v: u-preactivation of a padded column is ~0 -> h=c~0
    v = np.linalg.lstsq(ux_w.astype(np.float64),
                        -(ux_b + uh_b).astype(np.float64), rcond=None)[0]
    vcol = np.concatenate([v.astype(np.float32), [1.0]])    # row 300 = 1

    xT = np.concatenate([inputs.T.astype(np.float32),
                         np.ones((1, n), np.float32)], axis=0)  # [301, N]

    # --- weights, shared across cores ---
    Wp = np.zeros((302, 2 * H), np.float32)
    Wp[:D, :H] = ix_w.T
    Wp[:D, H:] = ux_w.T
    Wp[D, :H] = ix_b + ih_b
    Wp[D, H:] = ux_b + uh_b
    Wh = np.concatenate([ih_w.T, uh_w.T], axis=1)           # [256, 512]
    Wfi = np.zeros((302, H), np.float32)
    Wfi[:D] = fi_w.T
    Wfi[D] = fi_b + fh_b
    Wfh = np.ascontiguousarray(fh_w.T)                      # [256, 256]

    def pack_a(W, blocks):         # rows 0..255 -> [128, 128*len(blocks)*?]
        cols = []
        for c0 in blocks:
            for i in range(2):
                cols.append(W[128 * i:128 * (i + 1), c0:c0 + 128])
        return np.concatenate(cols, axis=1)

    def pack_b(W, blocks):         # rows 256..301 (+pad) -> [23, ...]
        cols = []
        for c0 in blocks:
            for i in range(2):
                cols.append(W[256 + 23 * i:256 + 23 * (i + 1), c0:c0 + 128])
        return np.concatenate(cols, axis=1)

    mt4 = [0, 128, 256, 384]
    mt2 = [0, 128]
    weights = {
        "wpa": _q8(pack_a(Wp, mt4)), "wpb": _q8(pack_b(Wp, mt4)),
        "wha": _q8(pack_a(Wh, mt4)),
        "wfha": _q8(pack_a(Wfh, mt2)),
        "wfia": _q8(pack_a(Wfi, mt2)), "wfib": _q8(pack_b(Wfi, mt2)),
        "wp_bf": _qb(Wp[:301]), "wh_bf": _qb(Wh),
        "wfi_bf": _qb(Wfi[:301]), "wfh_bf": _qb(Wfh),
        "xtop": _qb(xT[:, :CTOP]),
    }

    in_maps = []
    for g in range(N_CORES):
        xc = np.empty((302, CX), np.float32)
        xc[301] = 0.0
        js = 8 * np.arange(ST) + g
        # leaf block
        for s in range(SL):
            n0 = S8 + 256 * js[s]
            cnt = int(np.clip(n - n0, 0, 256))
            if cnt > 0:
                xc[:301, X8 + 256 * s:X8 + 256 * s + cnt] = xT[:, n0:n0 + cnt]
            if cnt < 256:
                xc[:301, X8 + 256 * s + cnt:X8 + 256 * (s + 1)] = \
                    vcol[:, None]
        # level blocks 7..4 (all real nodes)
        for (base, off, w) in ((S7, X7, 64), (S6, X6, 16),
                               (S5, X5, 4), (S4, X4, 1)):
            idx = (base + w * js[:, None] + np.arange(w)[None, :]).ravel()
            xc[:301, off:off + ST * w] = xT[:, idx]
        xc[:301, XT:] = xT[:, :CTOP]
        xa = _q8(np.concatenate([xc[0:128], xc[128:256]], axis=1))
        xb = _q8(np.concatenate([xc[256:279], xc[279:302]], axis=1))
        m = dict(weights)
        m["xa"] = xa
        m["xb"] = xb
        in_maps.append(m)
    return in_maps


def build_program(n=N_NODES, debug=False, timing=False):
    import concourse.bass as bass
    import concourse.tile as tile
    from concourse import bacc, mybir

    f32 = mybir.dt.float32
    bf16 = mybir.dt.bfloat16
    fp8 = mybir.dt.float8e4
    AF = mybir.ActivationFunctionType
    AX = mybir.AxisListType
    PM = mybir.MatmulPerfMode
    DR = PM.DoubleRow

    nc = bacc.Bacc("TRN2", target_bir_lowering=False, debug=debug,
                   num_devices=N_CORES)

    xa_d = nc.dram_tensor("xa", [P, 2 * CX], fp8, kind="ExternalInput")
    xb_d = nc.dram_tensor("xb", [23, 2 * CX], fp8, kind="ExternalInput")
    xtop_d = nc.dram_tensor("xtop", [301, CTOP], bf16, kind="ExternalInput")
    wpa_d = nc.dram_tensor("wpa", [P, 1024], fp8, kind="ExternalInput")
    wpb_d = nc.dram_tensor("wpb", [23, 1024], fp8, kind="ExternalInput")
    wha_d = nc.dram_tensor("wha", [P, 1024], fp8, kind="ExternalInput")
    wfha_d = nc.dram_tensor("wfha", [P, 512], fp8, kind="ExternalInput")
    wfia_d = nc.dram_tensor("wfia", [P, 512], fp8, kind="ExternalInput")
    wfib_d = nc.dram_tensor("wfib", [23, 512], fp8, kind="ExternalInput")
    wp_bf_d = nc.dram_tensor("wp_bf", [301, 512], bf16, kind="ExternalInput")
    wh_bf_d = nc.dram_tensor("wh_bf", [256, 512], bf16, kind="ExternalInput")
    wfi_bf_d = nc.dram_tensor("wfi_bf", [301, 256], bf16,
                              kind="ExternalInput")
    wfh_bf_d = nc.dram_tensor("wfh_bf", [256, 256], bf16,
                              kind="ExternalInput")
    h0_d = nc.dram_tensor("h0", [P, 2], f32, kind="ExternalOutput")
    c0_d = nc.dram_tensor("c0", [P, 2], f32, kind="ExternalOutput")

    with tile.TileContext(nc) as tc:
        import contextlib
        with contextlib.ExitStack() as stack:
            wpool = stack.enter_context(tc.tile_pool(name="w", bufs=1))
            state = stack.enter_context(tc.tile_pool(name="state", bufs=1))
            work = stack.enter_context(tc.tile_pool(name="work", bufs=2))
            psum = stack.enter_context(
                tc.tile_pool(name="psum", bufs=1, space="PSUM"))
            dram = stack.enter_context(
                tc.tile_pool(name="dram", bufs=1, space="DRAM"))

            # ---------------- weights ----------------
            wpa = wpool.tile([P, 1024], fp8, name="wpa")
            wpb = wpool.tile([23, 1024], fp8, name="wpb")
            wha = wpool.tile([P, 1024], fp8, name="wha")
            wfha = wpool.tile([P, 512], fp8, name="wfha")
            wfia = wpool.tile([P, 512], fp8, name="wfia")
            wfib = wpool.tile([23, 512], fp8, name="wfib")
            nc.sync.dma_start(wpa[:], wpa_d[:, :])
            nc.sync.dma_start(wpb[:], wpb_d[:, :])
            nc.sync.dma_start(wha[:], wha_d[:, :])
            nc.scalar.dma_start(wfha[:], wfha_d[:, :])
            nc.scalar.dma_start(wfia[:], wfia_d[:, :])
            nc.scalar.dma_start(wfib[:], wfib_d[:, :])
            # bf16 top weights, contraction chunks (0:128),(128:256),(256:301)
            wp_bf = [wpool.tile([128, 512], bf16, name="wp_bf0"),
                     wpool.tile([128, 512], bf16, name="wp_bf1"),
                     wpool.tile([45, 512], bf16, name="wp_bf2")]
            wfi_bf = [wpool.tile([128, 256], bf16, name="wfi_bf0"),
                      wpool.tile([128, 256], bf16, name="wfi_bf1"),
                      wpool.tile([45, 256], bf16, name="wfi_bf2")]
            wh_bf = [wpool.tile([128, 512], bf16, name="wh_bf0"),
                     wpool.tile([128, 512], bf16, name="wh_bf1")]
            wfh_bf = [wpool.tile([128, 256], bf16, name="wfh_bf0"),
                      wpool.tile([128, 256], bf16, name="wfh_bf1")]
            for k, (r0, r1) in enumerate(((0, 128), (128, 256), (256, 301))):
                nc.scalar.dma_start(wp_bf[k][:], wp_bf_d[r0:r1, :])
                nc.scalar.dma_start(wfi_bf[k][:], wfi_bf_d[r0:r1, :])
            for k in range(2):
                nc.scalar.dma_start(wh_bf[k][:], wh_bf_d[128 * k:128 * (k + 1), :])
                nc.scalar.dma_start(wfh_bf[k][:], wfh_bf_d[128 * k:128 * (k + 1), :])
            xt = [wpool.tile([128, CTOP], bf16, name="xt0"),
                  wpool.tile([128, CTOP], bf16, name="xt1"),
                  wpool.tile([45, CTOP], bf16, name="xt2")]
            for k, (r0, r1) in enumerate(((0, 128), (128, 256), (256, 301))):
                nc.scalar.dma_start(xt[k][:], xtop_d[r0:r1, :])

            # ---------------- x ----------------
            xa = state.tile([P, 2 * CX], fp8, name="xa")
            xb = state.tile([23, 2 * CX], fp8, name="xb")
            xav = xa[:].rearrange("k (two c) -> k two c", two=2)
            xbv = xb[:].rearrange("k (two c) -> k two c", two=2)
            xa_dv = xa_d[:, :].rearrange("k (two c) -> k two c", two=2)
            xb_dv = xb_d[:, :].rearrange("k (two c) -> k two c", two=2)
            for q, (c0, c1) in enumerate(((0, 1024), (1024, 3072),
                                          (3072, 5632), (5632, CX))):
                eng = nc.sync if q % 2 == 0 else nc.scalar
                eng.dma_start(xav[:, :, c0:c1], xa_dv[:, :, c0:c1])
            nc.scalar.dma_start(xbv[:, :, :], xb_dv[:, :, :])

            # ---------------- states ----------------
            h8 = state.tile([P, 2 * C8], fp8, name="h8")
            c8 = state.tile([P, 2 * C8], bf16, name="c8")
            h7 = state.tile([P, 2 * C7], fp8, name="h7")
            c7 = state.tile([P, 2 * C7], bf16, name="c7")
            h6 = state.tile([P, 2 * C6], fp8, name="h6")
            c6 = state.tile([P, 2 * C6], bf16, name="c6")
            h5 = state.tile([P, 2 * C5], fp8, name="h5")
            c5 = state.tile([P, 2 * C5], bf16, name="c5")
            h4f = state.tile([P, 2 * C4], bf16, name="h4f")
            c4f = state.tile([P, 2 * C4], bf16, name="c4f")
            hc4g = state.tile([P, 8 * 128], bf16, name="hc4g")
            h4g = state.tile([P, 2 * 256], bf16, name="h4g")
            c4g = state.tile([P, 2 * 256], bf16, name="c4g")
            h3 = state.tile([P, 2 * 64], bf16, name="h3")
            c3 = state.tile([P, 2 * 64], f32, name="c3")
            h2 = state.tile([P, 2 * 16], bf16, name="h2")
            c2 = state.tile([P, 2 * 16], f32, name="c2")
            h1 = state.tile([P, 2 * 4], bf16, name="h1")
            c1 = state.tile([P, 2 * 4], f32, name="c1")
            h0t = state.tile([P, 2], f32, name="h0t")
            c0t = state.tile([P, 2], f32, name="c0t")
            # per-level fc accumulators (bf16 except L4/top in f32)
            fc7 = state.tile([P, 2 * I7], bf16, name="fc7")
            fc6 = state.tile([P, 2 * C6], bf16, name="fc6")
            fc5 = state.tile([P, 2 * C5], bf16, name="fc5")
            fc4 = state.tile([P, 2 * C4], f32, name="fc4")
            fct = state.tile([P, 2 * 64], f32, name="fct")

            def sv(t, cols=0):     # state view [128, 2, c]
                return t[:].rearrange("k (two c) -> k two c", two=2)

            def wv_a(t, blk):      # fp8 DR lhsT view, A part
                return (t[:, 256 * blk:256 * (blk + 1)]
                        .rearrange("k (two m) -> k two m", two=2))

            def wv_b(t, blk):
                return (t[:, 256 * blk:256 * (blk + 1)]
                        .rearrange("k (two m) -> k two m", two=2))

            # ============ fp8 emitters ============

            def iu_chunk(xc0, L, tag, child=None, cc0=0):
                """i/u pre-acts for L parent cols at x cols [xc0, xc0+L).
                child=(h_tile, Ctot) adds the 4-child h sum (stride-4).
                Returns (pi, pu) psum tiles [P, 2*512] (cols 0:L, 512:512+L).
                """
                pi = psum.tile([P, 1024], f32, name=f"pi{tag}", tag="pi")
                pu = psum.tile([P, 1024], f32, name=f"pu{tag}", tag="pu")
                for gate, pt in ((0, pi), (1, pu)):
                    for mt in range(2):
                        out = pt[:, 512 * mt:512 * mt + L]
                        blk = 2 * gate + mt
                        mms = [(wv_a(wpa, blk), xav[:, :, xc0:xc0 + L], DR),
                               (wv_b(wpb, blk), xbv[:, :, xc0:xc0 + L], DR)]
                        if child is not None:
                            ht, Ct = child
                            hv = ht[:].rearrange("k (two c) -> k two c",
                                                 two=2)
                            for k in range(4):
                                mms.append(
                                    (wv_a(wha, blk),
                                     hv[:, :, cc0 + k:cc0 + 4 * L:4], DR))
                        for q, (w, r, pm) in enumerate(mms):
                            nc.tensor.matmul(out, w, r, start=(q == 0),
                                             stop=(q == len(mms) - 1),
                                             perf_mode=pm)
                return pi, pu

            def forget_chunk(lq0, Lf, xc0, tag, child_h, child_c, Cc, cc0,
                             fc_t, Mfc):
                """Forget path for Lf parents (<=256), x col xc0, children at
                child cols [cc0, cc0+4Lf). Writes fc_t cols [lq0, lq0+Lf) per
                mt (stride Mfc)."""
                hv = child_h[:].rearrange("k (two c) -> k two c", two=2)
                for mt in range(2):
                    pf = psum.tile([P, 1024], f32, name=f"pf{tag}_{mt}",
                                   tag="pf", bufs=2)
                    xva = (xav[:, :, xc0:xc0 + Lf].unsqueeze(3)
                           .broadcast_to([P, 2, Lf, 4]))
                    xvb = (xbv[:, :, xc0:xc0 + Lf].unsqueeze(3)
                           .broadcast_to([23, 2, Lf, 4]))
                    nc.tensor.matmul(pf[:, 0:4 * Lf], wv_a(wfha, mt),
                                     hv[:, :, cc0:cc0 + 4 * Lf],
                                     start=True, stop=False, perf_mode=DR)
                    nc.tensor.matmul(pf[:, 0:4 * Lf], wv_a(wfia, mt), xva,
                                     start=False, stop=False, perf_mode=DR)
                    nc.tensor.matmul(pf[:, 0:4 * Lf], wv_b(wfib, mt), xvb,
                                     start=False, stop=True, perf_mode=DR)
                    ft = work.tile([P, 1024], bf16, name=f"f{tag}_{mt}",
                                   tag="fM")
                    nc.scalar.activation(ft[:, 0:4 * Lf], pf[:, 0:4 * Lf],
                                         AF.Sigmoid)
                    fcc = work.tile([P, 1024], bf16, name=f"fx{tag}_{mt}",
                                    tag="fccM")
                    nc.vector.tensor_mul(
                        fcc[:, 0:4 * Lf], ft[:, 0:4 * Lf],
                        child_c[:, Cc * mt + cc0:Cc * mt + cc0 + 4 * Lf])
                    with nc.allow_low_precision(reason="fc bf16"):
                        nc.vector.reduce_sum(
                            fc_t[:, Mfc * mt + lq0:Mfc * mt + lq0 + Lf],
                            fcc[:, 0:4 * Lf]
                            .rearrange("k (l four) -> k l four", four=4),
                            axis=AX.X)

            def cio_chunk(pi, pu, L, tag, out_h, out_c, Cout, oc0,
                          fc_t=None, Mfc=None, lq0=0, h_dt_f32=False,
                          defer_h=False):
                """activations + c for L cols from iu psums; h = tanh(c)."""
                it = work.tile([P, 1024], bf16, name=f"i{tag}", tag="it")
                ut = work.tile([P, 1024], bf16, name=f"u{tag}", tag="ut")
                piv = pi[:].rearrange("k (mt c) -> k mt c", mt=2)[:, :, 0:L]
                puv = pu[:].rearrange("k (mt c) -> k mt c", mt=2)[:, :, 0:L]
                itv = it[:].rearrange("k (mt c) -> k mt c", mt=2)[:, :, 0:L]
                utv = ut[:].rearrange("k (mt c) -> k mt c", mt=2)[:, :, 0:L]
                nc.scalar.activation(itv, piv, AF.Sigmoid)
                nc.scalar.activation(utv, puv, AF.Tanh)
                ocv = sv(out_c, 0)[:, :, oc0:oc0 + L]
                if fc_t is None:
                    nc.vector.tensor_mul(ocv, itv, utv)
                else:
                    tt = work.tile([P, 1024], bf16, name=f"t{tag}", tag="tt")
                    ttv = (tt[:].rearrange("k (mt c) -> k mt c", mt=2)
                           [:, :, 0:L])
                    nc.vector.tensor_mul(ttv, itv, utv)
                    fcv = (fc_t[:].rearrange("k (mt c) -> k mt c", mt=2)
                           [:, :, lq0:lq0 + L])
                    nc.vector.tensor_add(ocv, ttv, fcv)
                if not defer_h:
                    emit_h(out_h, out_c, oc0, L, h_dt_f32)

            def emit_h(out_h, out_c, oc0, L, h_dt_f32=False):
                nc.scalar.activation(
                    sv(out_h, 0)[:, :, oc0:oc0 + L],
                    sv(out_c, 0)[:, :, oc0:oc0 + L], AF.Tanh)

            # ============ bf16 top path (batched x-pre) ============

            pre_i = state.tile([P, 2 * CTOP], f32, name="pre_i")
            pre_u = state.tile([P, 2 * CTOP], f32, name="pre_u")
            pre_fi = state.tile([P, 2 * CTOP], f32, name="pre_fi")

            def top_pre():
                """x-projections (i,u,fi) for all 85 replicated top nodes."""
                pi = psum.tile([P, 1024], f32, name="pitp", tag="pi")
                pu = psum.tile([P, 1024], f32, name="putp", tag="pu")
                for gate, pt in ((0, pi), (1, pu)):
                    for mt in range(2):
                        out = pt[:, 512 * mt:512 * mt + CTOP]
                        w0 = 256 * gate + 128 * mt
                        for k in range(3):
                            nc.tensor.matmul(out, wp_bf[k][:, w0:w0 + 128],
                                             xt[k][:, :], start=(k == 0),
                                             stop=(k == 2))
                pf = psum.tile([P, 1024], f32, name="pftp", tag="pf", bufs=2)
                for mt in range(2):
                    out = pf[:, 512 * mt:512 * mt + CTOP]
                    for k in range(3):
                        nc.tensor.matmul(
                            out, wfi_bf[k][:, 128 * mt:128 * mt + 128],
                            xt[k][:, :], start=(k == 0), stop=(k == 2))
                for dst, pt in ((pre_i, pi), (pre_u, pu), (pre_fi, pf)):
                    nc.vector.tensor_copy(
                        dst[:].rearrange("k (mt c) -> k mt c", mt=2),
                        pt[:].rearrange("k (mt c) -> k mt c", mt=2)
                        [:, :, 0:CTOP])

            def top_level(L, t0, ch_h, ch_c, out_h, out_c, li):
                """one replicated top level: h_sum on DVE, h-matmuls bf16,
                x parts from the pre-batch."""
                b = f"T{li}"
                chv = ch_h[:].rearrange("k (two c) -> k two c", two=2)
                # child h_sum
                hs = work.tile([P, 512], bf16, name=f"hs{b}", tag="hsT")
                hsv = hs[:].rearrange("k (two c) -> k two c", two=2)[:, :, 0:L]
                with nc.allow_low_precision(reason="hs bf16"):
                    nc.vector.reduce_sum(
                        hsv, chv[:, :, 0:4 * L]
                        .rearrange("k two (l four) -> k two l four", four=4),
                        axis=AX.X)
                # forget path
                pf = psum.tile([P, 1024], f32, name=f"pf{b}", tag="pf",
                               bufs=2)
                for mt in range(2):
                    for half in range(2):
                        nc.tensor.matmul(
                            pf[:, 512 * mt:512 * mt + 4 * L],
                            wfh_bf[half][:, 128 * mt:128 * mt + 128],
                            chv[:, half, 0:4 * L],
                            start=(half == 0), stop=(half == 1))
                fpre = work.tile([P, 1024], f32, name=f"fp{b}", tag="fpreT")
                fprev = (fpre[:].rearrange("k (mt c) -> k mt c", mt=2)
                         [:, :, 0:4 * L])
                fib = (pre_fi[:].rearrange("k (mt c) -> k mt c", mt=2)
                       [:, :, t0:t0 + L].unsqueeze(3)
                       .broadcast_to([P, 2, L, 4]))
                nc.vector.tensor_add(
                    fprev.rearrange("k mt (l four) -> k mt l four", four=4),
                    pf[:].rearrange("k (mt c) -> k mt c", mt=2)[:, :, 0:4 * L]
                    .rearrange("k mt (l four) -> k mt l four", four=4),
                    fib)
                ft = work.tile([P, 1024], bf16, name=f"f{b}", tag="fM")
                nc.scalar.activation(ft[:, 0:8 * L]
                                     .rearrange("k (mt c) -> k mt c", mt=2),
                                     fprev, AF.Sigmoid)
                fcc = work.tile([P, 1024], f32, name=f"fx{b}", tag="fccT")
                fccv = (fcc[:].rearrange("k (mt c) -> k mt c", mt=2)
                        [:, :, 0:4 * L])
                nc.vector.tensor_mul(
                    fccv, ft[:, 0:8 * L]
                    .rearrange("k (mt c) -> k mt c", mt=2),
                    ch_c[:].rearrange("k (two c) -> k two c", two=2)
                    [:, :, 0:4 * L])
                nc.vector.reduce_sum(
                    fct[:].rearrange("k (mt c) -> k mt c", mt=2)[:, :, 0:L],
                    fccv.rearrange("k mt (l four) -> k mt l four", four=4),
                    axis=AX.X)
                # i/u from h_sum matmuls + pre
                pi = psum.tile([P, 1024], f32, name=f"pi{b}", tag="pi")
                pu = psum.tile([P, 1024], f32, name=f"pu{b}", tag="pu")
                for gate, pt in ((0, pi), (1, pu)):
                    for mt in range(2):
                        w0 = 256 * gate + 128 * mt
                        for half in range(2):
                            nc.tensor.matmul(
                                pt[:, 512 * mt:512 * mt + L],
                                wh_bf[half][:, w0:w0 + 128],
                                hsv[:, half, :],
                                start=(half == 0), stop=(half == 1))
                ipre = work.tile([P, 1024], f32, name=f"ip{b}", tag="ppT")
                upre = work.tile([P, 1024], f32, name=f"up{b}", tag="ppT")
                for pre, pt, dst in ((pre_i, pi, ipre), (pre_u, pu, upre)):
                    nc.vector.tensor_add(
                        dst[:].rearrange("k (mt c) -> k mt c", mt=2)
                        [:, :, 0:L],
                        pt[:].rearrange("k (mt c) -> k mt c", mt=2)
                        [:, :, 0:L],
                        pre[:].rearrange("k (mt c) -> k mt c", mt=2)
                        [:, :, t0:t0 + L])
                it = work.tile([P, 1024], bf16, name=f"i{b}", tag="it")
                ut = work.tile([P, 1024], bf16, name=f"u{b}", tag="ut")
                itv = it[:].rearrange("k (mt c) -> k mt c", mt=2)[:, :, 0:L]
                utv = ut[:].rearrange("k (mt c) -> k mt c", mt=2)[:, :, 0:L]
                nc.scalar.activation(
                    itv, ipre[:].rearrange("k (mt c) -> k mt c", mt=2)
                    [:, :, 0:L], AF.Sigmoid)
                nc.scalar.activation(
                    utv, upre[:].rearrange("k (mt c) -> k mt c", mt=2)
                    [:, :, 0:L], AF.Tanh)
                tt = work.tile([P, 1024], f32, name=f"t{b}", tag="tt32")
                ttv = tt[:].rearrange("k (mt c) -> k mt c", mt=2)[:, :, 0:L]
                nc.vector.tensor_mul(ttv, itv, utv)
                ocv = sv(out_c)[:, :, 0:L]
                nc.vector.tensor_add(
                    ocv, ttv,
                    fct[:].rearrange("k (mt c) -> k mt c", mt=2)[:, :, 0:L])
                nc.scalar.activation(sv(out_h)[:, :, 0:L], ocv, AF.Tanh)

            # ============ emission schedule ============

            def leaf_chunk(j):     # 512 leaf cols
                c0 = 512 * j
                L = min(512, C8 - c0)
                pi, pu = iu_chunk(X8 + c0, L, f"l{j}")
                cio_chunk(pi, pu, L, f"l{j}", h8, c8, C8, c0, defer_h=True)

            def leaf_h(j):         # tanh over 1024 cols
                c0 = 1024 * j
                L = min(1024, C8 - c0)
                emit_h(h8, c8, c0, L)

            def tail_chunk(j):     # leaf-recipe level-7 cols
                c0 = I7 + 512 * j
                L = min(512, C7 - c0)
                pi, pu = iu_chunk(X7 + c0, L, f"t{j}")
                cio_chunk(pi, pu, L, f"t{j}", h7, c7, C7, c0)

            LVL = {
                7: dict(h=h7, c=c7, C=C7, fc=fc7, M=I7, xo=X7,
                        ch=h8, cc=c8, Cc=C8, npar=I7),
                6: dict(h=h6, c=c6, C=C6, fc=fc6, M=C6, xo=X6,
                        ch=h7, cc=c7, Cc=C7, npar=C6),
                5: dict(h=h5, c=c5, C=C5, fc=fc5, M=C5, xo=X5,
                        ch=h6, cc=c6, Cc=C6, npar=C5),
                4: dict(h=h4f, c=c4f, C=C4, fc=fc4, M=C4, xo=X4,
                        ch=h5, cc=c5, Cc=C5, npar=C4),
            }

            def fchunk(l, q):      # forget chunk q (256 parents) of level l
                v = LVL[l]
                q0 = 256 * q
                Lf = min(256, v["npar"] - q0)
                forget_chunk(q0, Lf, v["xo"] + q0, f"L{l}q{q}", v["ch"],
                             v["cc"], v["Cc"], 4 * q0, v["fc"], v["M"])

            def ichunk(l, j):      # iu+c+h chunk j (512 parents) of level l
                v = LVL[l]
                c0 = 512 * j
                L = min(512, v["npar"] - c0)
                pi, pu = iu_chunk(v["xo"] + c0, L, f"L{l}i{j}",
                                  child=(v["ch"], v["Cc"]), cc0=4 * c0)
                cio_chunk(pi, pu, L, f"L{l}i{j}", v["h"], v["c"], v["C"],
                          c0, fc_t=v["fc"], Mfc=v["M"], lq0=c0,
                          h_dt_f32=(l == 4))

            # --- interleaved schedule ---
            # leaf chunks: 11; tail: 2; L7: forget 6, iu 3; L6: f 2, iu 1;
            # L5: f 1, iu 1 (128); L4: f 1, iu 1 (32)
            # deps: leaf_h(q) <- leaf chunks 2q,2q+1;  fchunk(7,q) <- leaf_h(q)
            # ichunk(7,j) <- fchunk(7,2j),(7,2j+1) + leaf_h(2j),(2j+1)
            sched = [
                lambda: leaf_chunk(0), lambda: tail_chunk(0),
                lambda: top_pre(),
                lambda: leaf_chunk(1), lambda: tail_chunk(1),
                lambda: leaf_h(0),
                lambda: leaf_chunk(2), lambda: fchunk(7, 0),
                lambda: leaf_chunk(3), lambda: leaf_h(1),
                lambda: leaf_chunk(4), lambda: fchunk(7, 1),
                lambda: leaf_chunk(5), lambda: leaf_h(2),
                lambda: ichunk(7, 0),
                lambda: leaf_chunk(6), lambda: fchunk(7, 2),
                lambda: leaf_chunk(7), lambda: leaf_h(3),
                lambda: leaf_chunk(8), lambda: fchunk(7, 3),
                lambda: leaf_chunk(9), lambda: leaf_h(4),
                lambda: leaf_chunk(10), lambda: fchunk(7, 4),
                lambda: ichunk(7, 1), lambda: leaf_h(5),
                lambda: fchunk(7, 5), lambda: ichunk(7, 2),
                lambda: fchunk(6, 0), lambda: fchunk(6, 1),
                lambda: ichunk(6, 0),
                lambda: fchunk(5, 0), lambda: ichunk(5, 0),
                lambda: fchunk(4, 0), lambda: ichunk(4, 0),
            ]
            for fn in sched:
                fn()

            # ---- AllGather the 256 level-4 states (bf16, h|c packed) ----
            ag_in = dram.tile([1, 16384], bf16, name="ag_in")
            ag_out = dram.tile([N_CORES, 16384], bf16, name="ag_out")
            agv = ag_in[:].rearrange("o (p z) -> (o p) z", p=P)
            nc.sync.dma_start(agv[:, 0:64], h4f[:])
            nc.sync.dma_start(agv[:, 64:128], c4f[:])
            if timing:
                for g in range(N_CORES):
                    nc.sync.dma_start(ag_out[g:g + 1, :], ag_in[:])
            else:
                from concourse import mybir as _mb
                nc.gpsimd.collective_compute(
                    "AllGather", _mb.AluOpType.bypass,
                    replica_groups=[list(range(N_CORES))],
                    ins=[ag_in[:].opt()], outs=[ag_out[:].opt()])
            # single scatter DMA: (p, g*128+z) <- ag_out[g, p*128+z]
            nc.sync.dma_start(
                hc4g[:].rearrange("p (g z) -> p g z", g=N_CORES),
                ag_out[:, :].rearrange("g (p z) -> p g z", p=P))
            # unpermute to global order j = 8*s + g
            for dst, z0 in ((h4g, 0), (c4g, 64)):
                nc.vector.tensor_copy(
                    dst[:].rearrange("p (two s g) -> p two s g",
                                     two=2, s=32),
                    hc4g[:].rearrange("p (g z) -> p g z", g=N_CORES)
                    [:, :, z0:z0 + 64]
                    .rearrange("p g (two s) -> p two s g", two=2))

            # ---- replicated top: L3 (64), L2 (16), L1 (4), L0 (1) ----
            top_level(64, 21, h4g, c4g, h3, c3, 0)
            top_level(16, 5, h3, c3, h2, c2, 1)
            top_level(4, 1, h2, c2, h1, c1, 2)
            top_level(1, 0, h1, c1, h0t, c0t, 3)

            nc.sync.dma_start(h0_d[:, :], h0t[:])
            nc.sync.dma_start(c0_d[:, :], c0t[:])

    nc.compile()
    return nc


# ---------------------------------------------------------------------------
# self-contained entry point: kernel(**inputs) -> (h[0], c[0])
# ---------------------------------------------------------------------------
_CACHE = {}


def _ensure_paths():
    import sys
    for p in ("/opt/trn_rl_repo",):
        if p not in sys.path:
            sys.path.insert(0, p)


def _get_runner():
    if "runner" in _CACHE:
        return _CACHE["runner"]
    _ensure_paths()
    import jax
    from jax.sharding import Mesh, PartitionSpec, NamedSharding
    from jax.experimental.shard_map import shard_map
    from concourse import bass2jax, mybir

    nc = build_program(N_NODES)
    bass2jax.install_neuronx_cc_hook()
    partition_name = (nc.partition_id_tensor.name
                      if nc.partition_id_tensor else None)
    in_names, out_names, out_avals, zero_outs = [], [], [], []
    for alloc in nc.m.functions[0].allocations:
        if not isinstance(alloc, mybir.MemoryLocationSet):
            continue
        name = alloc.memorylocations[0].name
        if alloc.kind == "ExternalInput":
            if name != partition_name:
                in_names.append(name)
        elif alloc.kind == "ExternalOutput":
            out_names.append(name)
            shape = tuple(alloc.tensor_shape)
            dtype = mybir.dt.np(alloc.dtype)
            out_avals.append(jax.core.ShapedArray(shape, dtype))
            zero_outs.append(np.zeros(shape, dtype))
    all_in = list(in_names) + list(out_names)
    if partition_name is not None:
        all_in.append(partition_name)

    def _body(*args):
        operands = list(args)
        if partition_name is not None:
            operands.append(bass2jax.partition_id_tensor())
        return tuple(bass2jax._bass_exec_p.bind(
            *operands, out_avals=tuple(out_avals), in_names=tuple(all_in),
            out_names=tuple(out_names), lowering_input_output_aliases=(),
            sim_require_finite=True, sim_require_nnan=True, nc=nc))

    devices = jax.devices()[:N_CORES]
    assert len(devices) == N_CORES, (
        f"need {N_CORES} neuron devices, found {len(jax.devices())}")
    mesh = Mesh(np.asarray(devices), ("core",))
    nio = len(in_names) + len(out_names)
    sharded = jax.jit(
        shard_map(_body, mesh=mesh,
                  in_specs=(PartitionSpec("core"),) * nio,
                  out_specs=(PartitionSpec("core"),) * len(out_names),
                  check_rep=False),
        keep_unused=True)
    sh = NamedSharding(mesh, PartitionSpec("core"))
    runner = dict(run=sharded, in_names=in_names, out_names=out_names,
                  zero_outs=zero_outs, sh=sh, jax=jax)
    _CACHE["runner"] = runner
    return runner


def kernel(inputs, ix_w, ix_b, ih_w, ih_b, ux_w, ux_b, uh_w, uh_b,
           fi_w, fi_b, fh_w, fh_b):
    """ChildSum TreeLSTM over a complete 4-ary tree of 65536 nodes on 8
    NeuronCores (SPMD, one AllGather at the level-4 frontier).
    Returns (h[0], c[0]) as float32 arrays of shape (256,)."""
    assert np.asarray(inputs).shape == (N_NODES, D)
    in_maps = prep_inputs(
        np.asarray(inputs, np.float32),
        np.asarray(ix_w, np.float32), np.asarray(ix_b, np.float32),
        np.asarray(ih_w, np.float32), np.asarray(ih_b, np.float32),
        np.asarray(ux_w, np.float32), np.asarray(ux_b, np.float32),
        np.asarray(uh_w, np.float32), np.asarray(uh_b, np.float32),
        np.asarray(fi_w, np.float32), np.asarray(fi_b, np.float32),
        np.asarray(fh_w, np.float32), np.asarray(fh_b, np.float32))
    r = _get_runner()
    jax = r["jax"]
    concat = [np.concatenate([in_maps[c][nm] for c in range(N_CORES)], axis=0)
              for nm in r["in_names"]]
    dev_in = [jax.device_put(a, r["sh"]) for a in concat]
    dev_zero = [jax.device_put(
        np.zeros((N_CORES * z.shape[0], *z.shape[1:]), z.dtype), r["sh"])
        for z in r["zero_outs"]]
    outs = r["run"](*dev_in, *dev_zero)
    res = {nm: np.asarray(outs[i]).reshape(N_CORES, P, 2)[0]
           for i, nm in enumerate(r["out_names"])}
    h0 = res["h0"].T.reshape(2 * P).astype(np.float32)
    c0 = res["c0"].T.reshape(2 * P).astype(np.float32)
    return h0, c0


# revision 12
# speedup vs baseline: 1.3660x; 1.0086x over previous
"""ChildSum TreeLSTM on TRN2, 8-core SPMD Bass/Tile kernel — v3.

v3 vs v2 (178861 ns):
- L4-split sharding: each core owns 32 of the 256 level-4 subtrees
  (global subtree j = 8*s + core), so real leaves are balanced across
  cores: leaf block shrinks 8192 -> 5632 columns and the 5461 childless
  level-7 nodes run the cheap leaf recipe instead of the full gate path.
- fp8(e4m3) DoubleRow matmuls for the bulk: 2 contraction rows per
  partition at 0.5 cycles/output-column -> 4x the f32r rate.  x packed
  [128, 2, C] + [23, 2, C] (301 rows + zero pad = 128*2 + 23*2).
- fi folded into the forget-gate PSUM group via a broadcast (stride-0)
  rhs matmul -> the [P,4L] DVE add and fi copy disappear.
- child h_sum folded into the i/u PSUM group via 4 stride-4 child
  matmuls -> the DVE 4-way reduce disappears.
- elementwise in bf16 on DVE (2x mode), h states stored fp8 for the
  next level's DoubleRow matmuls, c states bf16.
- one AllGather of the 256 level-4 (h,c) states; levels 3..0 (85 nodes)
  replicated on every core in bf16.
"""

import numpy as np

D = 300
H = 256
P = 128
KB = 4
N_CORES = 8
N_NODES = 65536

ST = 32          # subtrees per core
SL = 22          # leaf-carrying subtree slots per core
C8 = SL * 256    # 5632 leaf columns
C7 = ST * 64     # 2048
I7 = SL * 64     # 1408 internal-recipe level-7 columns
T7 = C7 - I7     # 640 leaf-recipe level-7 columns
C6 = ST * 16     # 512
C5 = ST * 4      # 128
C4 = ST          # 32
CTOP = 85        # nodes 0..84 replicated
X8, X7, X6, X5, X4, XT = 0, 5632, 7680, 8192, 8320, 8352
CX = XT + CTOP   # 8437 x columns per core
S8, S7, S6, S5, S4 = 21845, 5461, 1365, 341, 85


def _q8(a):
    import ml_dtypes
    return np.asarray(a, dtype=ml_dtypes.float8_e4m3)


def _qb(a):
    import ml_dtypes
    return np.asarray(a, dtype=ml_dtypes.bfloat16)


def prep_inputs(inputs, ix_w, ix_b, ih_w, ih_b, ux_w, ux_b, uh_w, uh_b,
                fi_w, fi_b, fh_w, fh_b):
    n = inputs.shape[0]
    assert n == N_NODES

    # v: u-preactivation of a padded column is ~0 -> h=c~0
    v = np.linalg.lstsq(ux_w.astype(np.float64),
                        -(ux_b + uh_b).astype(np.float64), rcond=None)[0]
    vcol = np.concatenate([v.astype(np.float32), [1.0]])    # row 300 = 1

    xT = np.concatenate([inputs.T.astype(np.float32),
                         np.ones((1, n), np.float32)], axis=0)  # [301, N]

    # --- weights, shared across cores ---
    Wp = np.zeros((302, 2 * H), np.float32)
    Wp[:D, :H] = ix_w.T
    Wp[:D, H:] = ux_w.T
    Wp[D, :H] = ix_b + ih_b
    Wp[D, H:] = ux_b + uh_b
    Wh = np.concatenate([ih_w.T, uh_w.T], axis=1)           # [256, 512]
    Wfi = np.zeros((302, H), np.float32)
    Wfi[:D] = fi_w.T
    Wfi[D] = fi_b + fh_b
    Wfh = np.ascontiguousarray(fh_w.T)                      # [256, 256]

    def pack_a(W, blocks):         # rows 0..255 -> [128, 128*len(blocks)*?]
        cols = []
        for c0 in blocks:
            for i in range(2):
                cols.append(W[128 * i:128 * (i + 1), c0:c0 + 128])
        return np.concatenate(cols, axis=1)

    def pack_b(W, blocks):         # rows 256..301 (+pad) -> [23, ...]
        cols = []
        for c0 in blocks:
            for i in range(2):
                cols.append(W[256 + 23 * i:256 + 23 * (i + 1), c0:c0 + 128])
        return np.concatenate(cols, axis=1)

    mt4 = [0, 128, 256, 384]
    mt2 = [0, 128]
    weights = {
        "wpa": _q8(pack_a(Wp, mt4)), "wpb": _q8(pack_b(Wp, mt4)),
        "wha": _q8(pack_a(Wh, mt4)),
        "wfha": _q8(pack_a(Wfh, mt2)),
        "wfia": _q8(pack_a(Wfi, mt2)), "wfib": _q8(pack_b(Wfi, mt2)),
        "wp_bf": _qb(Wp[:301]), "wh_bf": _qb(Wh),
        "wfi_bf": _qb(Wfi[:301]), "wfh_bf": _qb(Wfh),
        "xtop": _qb(xT[:, :CTOP]),
    }

    in_maps = []
    for g in range(N_CORES):
        xc = np.empty((302, CX), np.float32)
        xc[301] = 0.0
        js = 8 * np.arange(ST) + g
        # leaf block
        for s in range(SL):
            n0 = S8 + 256 * js[s]
            cnt = int(np.clip(n - n0, 0, 256))
            if cnt > 0:
                xc[:301, X8 + 256 * s:X8 + 256 * s + cnt] = xT[:, n0:n0 + cnt]
            if cnt < 256:
                xc[:301, X8 + 256 * s + cnt:X8 + 256 * (s + 1)] = \
                    vcol[:, None]
        # level blocks 7..4 (all real nodes)
        for (base, off, w) in ((S7, X7, 64), (S6, X6, 16),
                               (S5, X5, 4), (S4, X4, 1)):
            idx = (base + w * js[:, None] + np.arange(w)[None, :]).ravel()
            xc[:301, off:off + ST * w] = xT[:, idx]
        xc[:301, XT:] = xT[:, :CTOP]
        xa = _q8(np.concatenate([xc[0:128], xc[128:256]], axis=1))
        xb = _q8(np.concatenate([xc[256:279], xc[279:302]], axis=1))
        m = dict(weights)
        m["xa"] = xa
        m["xb"] = xb
        in_maps.append(m)
    return in_maps


def build_program(n=N_NODES, debug=False, timing=False):
    import concourse.bass as bass
    import concourse.tile as tile
    from concourse import bacc, mybir

    f32 = mybir.dt.float32
    bf16 = mybir.dt.bfloat16
    fp8 = mybir.dt.float8e4
    AF = mybir.ActivationFunctionType
    AX = mybir.AxisListType
    PM = mybir.MatmulPerfMode
    DR = PM.DoubleRow

    nc = bacc.Bacc("TRN2", target_bir_lowering=False, debug=debug,
                   num_devices=N_CORES)

    xa_d = nc.dram_tensor("xa", [P, 2 * CX], fp8, kind="ExternalInput")
    xb_d = nc.dram_tensor("xb", [23, 2 * CX], fp8, kind="ExternalInput")
    xtop_d = nc.dram_tensor("xtop", [301, CTOP], bf16, kind="ExternalInput")
    wpa_d = nc.dram_tensor("wpa", [P, 1024], fp8, kind="ExternalInput")
    wpb_d = nc.dram_tensor("wpb", [23, 1024], fp8, kind="ExternalInput")
    wha_d = nc.dram_tensor("wha", [P, 1024], fp8, kind="ExternalInput")
    wfha_d = nc.dram_tensor("wfha", [P, 512], fp8, kind="ExternalInput")
    wfia_d = nc.dram_tensor("wfia", [P, 512], fp8, kind="ExternalInput")
    wfib_d = nc.dram_tensor("wfib", [23, 512], fp8, kind="ExternalInput")
    wp_bf_d = nc.dram_tensor("wp_bf", [301, 512], bf16, kind="ExternalInput")
    wh_bf_d = nc.dram_tensor("wh_bf", [256, 512], bf16, kind="ExternalInput")
    wfi_bf_d = nc.dram_tensor("wfi_bf", [301, 256], bf16,
                              kind="ExternalInput")
    wfh_bf_d = nc.dram_tensor("wfh_bf", [256, 256], bf16,
                              kind="ExternalInput")
    h0_d = nc.dram_tensor("h0", [P, 2], f32, kind="ExternalOutput")
    c0_d = nc.dram_tensor("c0", [P, 2], f32, kind="ExternalOutput")

    with tile.TileContext(nc) as tc:
        import contextlib
        with contextlib.ExitStack() as stack:
            wpool = stack.enter_context(tc.tile_pool(name="w", bufs=1))
            state = stack.enter_context(tc.tile_pool(name="state", bufs=1))
            work = stack.enter_context(tc.tile_pool(name="work", bufs=2))
            psum = stack.enter_context(
                tc.tile_pool(name="psum", bufs=1, space="PSUM"))
            dram = stack.enter_context(
                tc.tile_pool(name="dram", bufs=1, space="DRAM"))

            # ---------------- weights ----------------
            wpa = wpool.tile([P, 1024], fp8, name="wpa")
            wpb = wpool.tile([23, 1024], fp8, name="wpb")
            wha = wpool.tile([P, 1024], fp8, name="wha")
            wfha = wpool.tile([P, 512], fp8, name="wfha")
            wfia = wpool.tile([P, 512], fp8, name="wfia")
            wfib = wpool.tile([23, 512], fp8, name="wfib")
            nc.sync.dma_start(wpa[:], wpa_d[:, :])
            nc.sync.dma_start(wpb[:], wpb_d[:, :])
            nc.sync.dma_start(wha[:], wha_d[:, :])
            nc.scalar.dma_start(wfha[:], wfha_d[:, :])
            nc.scalar.dma_start(wfia[:], wfia_d[:, :])
            nc.scalar.dma_start(wfib[:], wfib_d[:, :])
            # bf16 top weights, contraction chunks (0:128),(128:256),(256:301)
            wp_bf = [wpool.tile([128, 512], bf16, name="wp_bf0"),
                     wpool.tile([128, 512], bf16, name="wp_bf1"),
                     wpool.tile([45, 512], bf16, name="wp_bf2")]
            wfi_bf = [wpool.tile([128, 256], bf16, name="wfi_bf0"),
                      wpool.tile([128, 256], bf16, name="wfi_bf1"),
                      wpool.tile([45, 256], bf16, name="wfi_bf2")]
            wh_bf = [wpool.tile([128, 512], bf16, name="wh_bf0"),
                     wpool.tile([128, 512], bf16, name="wh_bf1")]
            wfh_bf = [wpool.tile([128, 256], bf16, name="wfh_bf0"),
                      wpool.tile([128, 256], bf16, name="wfh_bf1")]
            for k, (r0, r1) in enumerate(((0, 128), (128, 256), (256, 301))):
                nc.scalar.dma_start(wp_bf[k][:], wp_bf_d[r0:r1, :])
                nc.scalar.dma_start(wfi_bf[k][:], wfi_bf_d[r0:r1, :])
            for k in range(2):
                nc.scalar.dma_start(wh_bf[k][:], wh_bf_d[128 * k:128 * (k + 1), :])
                nc.scalar.dma_start(wfh_bf[k][:], wfh_bf_d[128 * k:128 * (k + 1), :])
            xt = [wpool.tile([128, CTOP], bf16, name="xt0"),
                  wpool.tile([128, CTOP], bf16, name="xt1"),
                  wpool.tile([45, CTOP], bf16, name="xt2")]
            for k, (r0, r1) in enumerate(((0, 128), (128, 256), (256, 301))):
                nc.scalar.dma_start(xt[k][:], xtop_d[r0:r1, :])

            # ---------------- x ----------------
            xa = state.tile([P, 2 * CX], fp8, name="xa")
            xb = state.tile([23, 2 * CX], fp8, name="xb")
            xav = xa[:].rearrange("k (two c) -> k two c", two=2)
            xbv = xb[:].rearrange("k (two c) -> k two c", two=2)
            xa_dv = xa_d[:, :].rearrange("k (two c) -> k two c", two=2)
            xb_dv = xb_d[:, :].rearrange("k (two c) -> k two c", two=2)
            for q, (c0, c1) in enumerate(((0, 1024), (1024, 3072),
                                          (3072, 5632), (5632, CX))):
                eng = nc.sync if q % 2 == 0 else nc.scalar
                eng.dma_start(xav[:, :, c0:c1], xa_dv[:, :, c0:c1])
            nc.scalar.dma_start(xbv[:, :, :], xb_dv[:, :, :])

            # ---------------- states ----------------
            h8 = state.tile([P, 2 * C8], fp8, name="h8")
            c8 = state.tile([P, 2 * C8], bf16, name="c8")
            h7 = state.tile([P, 2 * C7], fp8, name="h7")
            c7 = state.tile([P, 2 * C7], bf16, name="c7")
            h6 = state.tile([P, 2 * C6], fp8, name="h6")
            c6 = state.tile([P, 2 * C6], bf16, name="c6")
            h5 = state.tile([P, 2 * C5], fp8, name="h5")
            c5 = state.tile([P, 2 * C5], bf16, name="c5")
            h4f = state.tile([P, 2 * C4], bf16, name="h4f")
            c4f = state.tile([P, 2 * C4], bf16, name="c4f")
            hc4g = state.tile([P, 8 * 128], bf16, name="hc4g")
            h4g = state.tile([P, 2 * 256], bf16, name="h4g")
            c4g = state.tile([P, 2 * 256], bf16, name="c4g")
            h3 = state.tile([P, 2 * 64], bf16, name="h3")
            c3 = state.tile([P, 2 * 64], f32, name="c3")
            h2 = state.tile([P, 2 * 16], bf16, name="h2")
            c2 = state.tile([P, 2 * 16], f32, name="c2")
            h1 = state.tile([P, 2 * 4], bf16, name="h1")
            c1 = state.tile([P, 2 * 4], f32, name="c1")
            h0t = state.tile([P, 2], f32, name="h0t")
            c0t = state.tile([P, 2], f32, name="c0t")
            # per-level fc accumulators (bf16 except L4/top in f32)
            fc7 = state.tile([P, 2 * I7], bf16, name="fc7")
            fc6 = state.tile([P, 2 * C6], bf16, name="fc6")
            fc5 = state.tile([P, 2 * C5], bf16, name="fc5")
            fc4 = state.tile([P, 2 * C4], f32, name="fc4")
            fct = state.tile([P, 2 * 64], f32, name="fct")

            def sv(t, cols=0):     # state view [128, 2, c]
                return t[:].rearrange("k (two c) -> k two c", two=2)

            def wv_a(t, blk):      # fp8 DR lhsT view, A part
                return (t[:, 256 * blk:256 * (blk + 1)]
                        .rearrange("k (two m) -> k two m", two=2))

            def wv_b(t, blk):
                return (t[:, 256 * blk:256 * (blk + 1)]
                        .rearrange("k (two m) -> k two m", two=2))

            # ============ fp8 emitters ============

            def iu_chunk(xc0, L, tag, child=None, cc0=0):
                """i/u pre-acts for L parent cols at x cols [xc0, xc0+L).
                child=(h_tile, Ctot) adds the 4-child h sum (stride-4).
                Returns (pi, pu) psum tiles [P, 2*512] (cols 0:L, 512:512+L).
                """
                pi = psum.tile([P, 1024], f32, name=f"pi{tag}", tag="pi")
                pu = psum.tile([P, 1024], f32, name=f"pu{tag}", tag="pu")
                for gate, pt in ((0, pi), (1, pu)):
                    for mt in range(2):
                        out = pt[:, 512 * mt:512 * mt + L]
                        blk = 2 * gate + mt
                        mms = [(wv_a(wpa, blk), xav[:, :, xc0:xc0 + L], DR),
                               (wv_b(wpb, blk), xbv[:, :, xc0:xc0 + L], DR)]
                        if child is not None:
                            ht, Ct = child
                            hv = ht[:].rearrange("k (two c) -> k two c",
                                                 two=2)
                            for k in range(4):
                                mms.append(
                                    (wv_a(wha, blk),
                                     hv[:, :, cc0 + k:cc0 + 4 * L:4], DR))
                        for q, (w, r, pm) in enumerate(mms):
                            nc.tensor.matmul(out, w, r, start=(q == 0),
                                             stop=(q == len(mms) - 1),
                                             perf_mode=pm)
                return pi, pu

            def forget_chunk(lq0, Lf, xc0, tag, child_h, child_c, Cc, cc0,
                             fc_t, Mfc):
                """Forget path for Lf parents (<=256), x col xc0, children at
                child cols [cc0, cc0+4Lf). Writes fc_t cols [lq0, lq0+Lf) per
                mt (stride Mfc)."""
                hv = child_h[:].rearrange("k (two c) -> k two c", two=2)
                for mt in range(2):
                    pf = psum.tile([P, 512], f32, name=f"pf{tag}_{mt}",
                                   tag="pf", bufs=4)
                    xva = (xav[:, :, xc0:xc0 + Lf].unsqueeze(3)
                           .broadcast_to([P, 2, Lf, 4]))
                    xvb = (xbv[:, :, xc0:xc0 + Lf].unsqueeze(3)
                           .broadcast_to([23, 2, Lf, 4]))
                    nc.tensor.matmul(pf[:, 0:4 * Lf], wv_a(wfha, mt),
                                     hv[:, :, cc0:cc0 + 4 * Lf],
                                     start=True, stop=False, perf_mode=DR)
                    nc.tensor.matmul(pf[:, 0:4 * Lf], wv_a(wfia, mt), xva,
                                     start=False, stop=False, perf_mode=DR)
                    nc.tensor.matmul(pf[:, 0:4 * Lf], wv_b(wfib, mt), xvb,
                                     start=False, stop=True, perf_mode=DR)
                    ft = work.tile([P, 1024], bf16, name=f"f{tag}_{mt}",
                                   tag="fM")
                    nc.scalar.activation(ft[:, 0:4 * Lf], pf[:, 0:4 * Lf],
                                         AF.Sigmoid)
                    fcc = work.tile([P, 1024], bf16, name=f"fx{tag}_{mt}",
                                    tag="fccM")
                    nc.vector.tensor_mul(
                        fcc[:, 0:4 * Lf], ft[:, 0:4 * Lf],
                        child_c[:, Cc * mt + cc0:Cc * mt + cc0 + 4 * Lf])
                    with nc.allow_low_precision(reason="fc bf16"):
                        nc.vector.reduce_sum(
                            fc_t[:, Mfc * mt + lq0:Mfc * mt + lq0 + Lf],
                            fcc[:, 0:4 * Lf]
                            .rearrange("k (l four) -> k l four", four=4),
                            axis=AX.X)

            def cio_chunk(pi, pu, L, tag, out_h, out_c, Cout, oc0,
                          fc_t=None, Mfc=None, lq0=0, h_dt_f32=False,
                          defer_h=False):
                """activations + c for L cols from iu psums; h = tanh(c)."""
                it = work.tile([P, 1024], bf16, name=f"i{tag}", tag="it")
                ut = work.tile([P, 1024], bf16, name=f"u{tag}", tag="ut")
                piv = pi[:].rearrange("k (mt c) -> k mt c", mt=2)[:, :, 0:L]
                puv = pu[:].rearrange("k (mt c) -> k mt c", mt=2)[:, :, 0:L]
                itv = it[:].rearrange("k (mt c) -> k mt c", mt=2)[:, :, 0:L]
                utv = ut[:].rearrange("k (mt c) -> k mt c", mt=2)[:, :, 0:L]
                nc.scalar.activation(itv, piv, AF.Sigmoid)
                nc.scalar.activation(utv, puv, AF.Tanh)
                ocv = sv(out_c, 0)[:, :, oc0:oc0 + L]
                if fc_t is None:
                    nc.vector.tensor_mul(ocv, itv, utv)
                else:
                    tt = work.tile([P, 1024], bf16, name=f"t{tag}", tag="tt")
                    ttv = (tt[:].rearrange("k (mt c) -> k mt c", mt=2)
                           [:, :, 0:L])
                    nc.vector.tensor_mul(ttv, itv, utv)
                    fcv = (fc_t[:].rearrange("k (mt c) -> k mt c", mt=2)
                           [:, :, lq0:lq0 + L])
                    nc.vector.tensor_add(ocv, ttv, fcv)
                if not defer_h:
                    emit_h(out_h, out_c, oc0, L, h_dt_f32)

            def emit_h(out_h, out_c, oc0, L, h_dt_f32=False):
                nc.scalar.activation(
                    sv(out_h, 0)[:, :, oc0:oc0 + L],
                    sv(out_c, 0)[:, :, oc0:oc0 + L], AF.Tanh)

            # ============ bf16 top path (batched x-pre) ============

            pre_i = state.tile([P, 2 * CTOP], f32, name="pre_i")
            pre_u = state.tile([P, 2 * CTOP], f32, name="pre_u")
            pre_fi = state.tile([P, 2 * CTOP], f32, name="pre_fi")

            def top_pre():
                """x-projections (i,u,fi) for all 85 replicated top nodes."""
                pi = psum.tile([P, 1024], f32, name="pitp", tag="pi")
                pu = psum.tile([P, 1024], f32, name="putp", tag="pu")
                for gate, pt in ((0, pi), (1, pu)):
                    for mt in range(2):
                        out = pt[:, 512 * mt:512 * mt + CTOP]
                        w0 = 256 * gate + 128 * mt
                        for k in range(3):
                            nc.tensor.matmul(out, wp_bf[k][:, w0:w0 + 128],
                                             xt[k][:, :], start=(k == 0),
                                             stop=(k == 2))
                pf = psum.tile([P, 512], f32, name="pftp", tag="pf",
                               bufs=4)
                for mt in range(2):
                    out = pf[:, 256 * mt:256 * mt + CTOP]
                    for k in range(3):
                        nc.tensor.matmul(
                            out, wfi_bf[k][:, 128 * mt:128 * mt + 128],
                            xt[k][:, :], start=(k == 0), stop=(k == 2))
                for dst, pt in ((pre_i, pi), (pre_u, pu), (pre_fi, pf)):
                    nc.vector.tensor_copy(
                        dst[:].rearrange("k (mt c) -> k mt c", mt=2),
                        pt[:].rearrange("k (mt c) -> k mt c", mt=2)
                        [:, :, 0:CTOP])

            def top_level(L, t0, ch_h, ch_c, out_h, out_c, li):
                """one replicated top level: h_sum on DVE, h-matmuls bf16,
                x parts from the pre-batch."""
                b = f"T{li}"
                chv = ch_h[:].rearrange("k (two c) -> k two c", two=2)
                # child h_sum
                hs = work.tile([P, 512], bf16, name=f"hs{b}", tag="hsT")
                hsv = hs[:].rearrange("k (two c) -> k two c", two=2)[:, :, 0:L]
                with nc.allow_low_precision(reason="hs bf16"):
                    nc.vector.reduce_sum(
                        hsv, chv[:, :, 0:4 * L]
                        .rearrange("k two (l four) -> k two l four", four=4),
                        axis=AX.X)
                # forget path
                pf = psum.tile([P, 512], f32, name=f"pf{b}", tag="pf",
                               bufs=4)
                for mt in range(2):
                    for half in range(2):
                        nc.tensor.matmul(
                            pf[:, 256 * mt:256 * mt + 4 * L],
                            wfh_bf[half][:, 128 * mt:128 * mt + 128],
                            chv[:, half, 0:4 * L],
                            start=(half == 0), stop=(half == 1))
                fpre = work.tile([P, 1024], f32, name=f"fp{b}", tag="fpreT")
                fprev = (fpre[:].rearrange("k (mt c) -> k mt c", mt=2)
                         [:, :, 0:4 * L])
                fib = (pre_fi[:].rearrange("k (mt c) -> k mt c", mt=2)
                       [:, :, t0:t0 + L].unsqueeze(3)
                       .broadcast_to([P, 2, L, 4]))
                nc.vector.tensor_add(
                    fprev.rearrange("k mt (l four) -> k mt l four", four=4),
                    pf[:].rearrange("k (mt c) -> k mt c", mt=2)[:, :, 0:4 * L]
                    .rearrange("k mt (l four) -> k mt l four", four=4),
                    fib)
                ft = work.tile([P, 1024], bf16, name=f"f{b}", tag="fM")
                nc.scalar.activation(ft[:, 0:8 * L]
                                     .rearrange("k (mt c) -> k mt c", mt=2),
                                     fprev, AF.Sigmoid)
                fcc = work.tile([P, 1024], f32, name=f"fx{b}", tag="fccT")
                fccv = (fcc[:].rearrange("k (mt c) -> k mt c", mt=2)
                        [:, :, 0:4 * L])
                nc.vector.tensor_mul(
                    fccv, ft[:, 0:8 * L]
                    .rearrange("k (mt c) -> k mt c", mt=2),
                    ch_c[:].rearrange("k (two c) -> k two c", two=2)
                    [:, :, 0:4 * L])
                nc.vector.reduce_sum(
                    fct[:].rearrange("k (mt c) -> k mt c", mt=2)[:, :, 0:L],
                    fccv.rearrange("k mt (l four) -> k mt l four", four=4),
                    axis=AX.X)
                # i/u from h_sum matmuls + pre
                pi = psum.tile([P, 1024], f32, name=f"pi{b}", tag="pi")
                pu = psum.tile([P, 1024], f32, name=f"pu{b}", tag="pu")
                for gate, pt in ((0, pi), (1, pu)):
                    for mt in range(2):
                        w0 = 256 * gate + 128 * mt
                        for half in range(2):
                            nc.tensor.matmul(
                                pt[:, 512 * mt:512 * mt + L],
                                wh_bf[half][:, w0:w0 + 128],
                                hsv[:, half, :],
                                start=(half == 0), stop=(half == 1))
                ipre = work.tile([P, 1024], f32, name=f"ip{b}", tag="ppT")
                upre = work.tile([P, 1024], f32, name=f"up{b}", tag="ppT")
                for pre, pt, dst in ((pre_i, pi, ipre), (pre_u, pu, upre)):
                    nc.vector.tensor_add(
                        dst[:].rearrange("k (mt c) -> k mt c", mt=2)
                        [:, :, 0:L],
                        pt[:].rearrange("k (mt c) -> k mt c", mt=2)
                        [:, :, 0:L],
                        pre[:].rearrange("k (mt c) -> k mt c", mt=2)
                        [:, :, t0:t0 + L])
                it = work.tile([P, 1024], bf16, name=f"i{b}", tag="it")
                ut = work.tile([P, 1024], bf16, name=f"u{b}", tag="ut")
                itv = it[:].rearrange("k (mt c) -> k mt c", mt=2)[:, :, 0:L]
                utv = ut[:].rearrange("k (mt c) -> k mt c", mt=2)[:, :, 0:L]
                nc.scalar.activation(
                    itv, ipre[:].rearrange("k (mt c) -> k mt c", mt=2)
                    [:, :, 0:L], AF.Sigmoid)
                nc.scalar.activation(
                    utv, upre[:].rearrange("k (mt c) -> k mt c", mt=2)
                    [:, :, 0:L], AF.Tanh)
                tt = work.tile([P, 1024], f32, name=f"t{b}", tag="tt32")
                ttv = tt[:].rearrange("k (mt c) -> k mt c", mt=2)[:, :, 0:L]
                nc.vector.tensor_mul(ttv, itv, utv)
                ocv = sv(out_c)[:, :, 0:L]
                nc.vector.tensor_add(
                    ocv, ttv,
                    fct[:].rearrange("k (mt c) -> k mt c", mt=2)[:, :, 0:L])
                nc.scalar.activation(sv(out_h)[:, :, 0:L], ocv, AF.Tanh)

            # ============ emission schedule ============

            def leaf_chunk(j):     # 512 leaf cols
                c0 = 512 * j
                L = min(512, C8 - c0)
                pi, pu = iu_chunk(X8 + c0, L, f"l{j}")
                cio_chunk(pi, pu, L, f"l{j}", h8, c8, C8, c0, defer_h=True)

            def leaf_h(j):         # tanh over 1024 cols
                c0 = 1024 * j
                L = min(1024, C8 - c0)
                emit_h(h8, c8, c0, L)

            def tail_chunk(j):     # leaf-recipe level-7 cols
                c0 = I7 + 512 * j
                L = min(512, C7 - c0)
                pi, pu = iu_chunk(X7 + c0, L, f"t{j}")
                cio_chunk(pi, pu, L, f"t{j}", h7, c7, C7, c0)

            LVL = {
                7: dict(h=h7, c=c7, C=C7, fc=fc7, M=I7, xo=X7,
                        ch=h8, cc=c8, Cc=C8, npar=I7),
                6: dict(h=h6, c=c6, C=C6, fc=fc6, M=C6, xo=X6,
                        ch=h7, cc=c7, Cc=C7, npar=C6),
                5: dict(h=h5, c=c5, C=C5, fc=fc5, M=C5, xo=X5,
                        ch=h6, cc=c6, Cc=C6, npar=C5),
                4: dict(h=h4f, c=c4f, C=C4, fc=fc4, M=C4, xo=X4,
                        ch=h5, cc=c5, Cc=C5, npar=C4),
            }

            def fchunk(l, q):      # forget chunk q (128 parents) of level l
                v = LVL[l]
                q0 = 128 * q
                Lf = min(128, v["npar"] - q0)
                forget_chunk(q0, Lf, v["xo"] + q0, f"L{l}q{q}", v["ch"],
                             v["cc"], v["Cc"], 4 * q0, v["fc"], v["M"])

            def ichunk(l, j):      # iu+c+h chunk j (512 parents) of level l
                v = LVL[l]
                c0 = 512 * j
                L = min(512, v["npar"] - c0)
                pi, pu = iu_chunk(v["xo"] + c0, L, f"L{l}i{j}",
                                  child=(v["ch"], v["Cc"]), cc0=4 * c0)
                cio_chunk(pi, pu, L, f"L{l}i{j}", v["h"], v["c"], v["C"],
                          c0, fc_t=v["fc"], Mfc=v["M"], lq0=c0,
                          h_dt_f32=(l == 4))

            # --- interleaved schedule ---
            # leaf chunks: 11; tail: 2; L7: forget 6, iu 3; L6: f 2, iu 1;
            # L5: f 1, iu 1 (128); L4: f 1, iu 1 (32)
            # deps: leaf_h(q) <- leaf chunks 2q,2q+1
            # fchunk(7,q): children leaf cols 512q..512q+512 <- leaf_h(q//2)
            # ichunk(7,j): fc cols 512j..512j+512 <- fchunk(7,4j..4j+3)
            sched = [
                lambda: leaf_chunk(0), lambda: tail_chunk(0),
                lambda: leaf_chunk(1), lambda: tail_chunk(1),
                lambda: top_pre(), lambda: leaf_h(0),
                lambda: leaf_chunk(2), lambda: fchunk(7, 0),
                lambda: leaf_chunk(3), lambda: fchunk(7, 1),
                lambda: leaf_h(1),
                lambda: leaf_chunk(4), lambda: fchunk(7, 2),
                lambda: leaf_chunk(5), lambda: fchunk(7, 3),
                lambda: leaf_h(2), lambda: ichunk(7, 0),
                lambda: leaf_chunk(6), lambda: fchunk(7, 4),
                lambda: leaf_chunk(7), lambda: fchunk(7, 5),
                lambda: leaf_h(3),
                lambda: leaf_chunk(8), lambda: fchunk(7, 6),
                lambda: leaf_chunk(9), lambda: fchunk(7, 7),
                lambda: leaf_h(4), lambda: ichunk(7, 1),
                lambda: leaf_chunk(10), lambda: fchunk(7, 8),
                lambda: leaf_h(5), lambda: fchunk(7, 9),
                lambda: fchunk(7, 10), lambda: ichunk(7, 2),
                lambda: fchunk(6, 0), lambda: fchunk(6, 1),
                lambda: fchunk(6, 2), lambda: fchunk(6, 3),
                lambda: ichunk(6, 0),
                lambda: fchunk(5, 0), lambda: ichunk(5, 0),
                lambda: fchunk(4, 0), lambda: ichunk(4, 0),
            ]
            for fn in sched:
                fn()

            # ---- AllGather the 256 level-4 states (bf16, h|c packed) ----
            ag_in = dram.tile([1, 16384], bf16, name="ag_in")
            ag_out = dram.tile([N_CORES, 16384], bf16, name="ag_out")
            agv = ag_in[:].rearrange("o (p z) -> (o p) z", p=P)
            nc.sync.dma_start(agv[:, 0:64], h4f[:])
            nc.sync.dma_start(agv[:, 64:128], c4f[:])
            if timing:
                for g in range(N_CORES):
                    nc.sync.dma_start(ag_out[g:g + 1, :], ag_in[:])
            else:
                from concourse import mybir as _mb
                nc.gpsimd.collective_compute(
                    "AllGather", _mb.AluOpType.bypass,
                    replica_groups=[list(range(N_CORES))],
                    ins=[ag_in[:].opt()], outs=[ag_out[:].opt()])
            # single scatter DMA: (p, g*128+z) <- ag_out[g, p*128+z]
            nc.sync.dma_start(
                hc4g[:].rearrange("p (g z) -> p g z", g=N_CORES),
                ag_out[:, :].rearrange("g (p z) -> p g z", p=P))
            # unpermute to global order j = 8*s + g
            for dst, z0 in ((h4g, 0), (c4g, 64)):
                nc.vector.tensor_copy(
                    dst[:].rearrange("p (two s g) -> p two s g",
                                     two=2, s=32),
                    hc4g[:].rearrange("p (g z) -> p g z", g=N_CORES)
                    [:, :, z0:z0 + 64]
                    .rearrange("p g (two s) -> p two s g", two=2))

            # ---- replicated top: L3 (64), L2 (16), L1 (4), L0 (1) ----
            top_level(64, 21, h4g, c4g, h3, c3, 0)
            top_level(16, 5, h3, c3, h2, c2, 1)
            top_level(4, 1, h2, c2, h1, c1, 2)
            top_level(1, 0, h1, c1, h0t, c0t, 3)

            nc.sync.dma_start(h0_d[:, :], h0t[:])
            nc.sync.dma_start(c0_d[:, :], c0t[:])

    nc.compile()
    return nc


# ---------------------------------------------------------------------------
# self-contained entry point: kernel(**inputs) -> (h[0], c[0])
# ---------------------------------------------------------------------------
_CACHE = {}


def _ensure_paths():
    import sys
    for p in ("/opt/trn_rl_repo",):
        if p not in sys.path:
            sys.path.insert(0, p)


def _get_runner():
    if "runner" in _CACHE:
        return _CACHE["runner"]
    _ensure_paths()
    import jax
    from jax.sharding import Mesh, PartitionSpec, NamedSharding
    from jax.experimental.shard_map import shard_map
    from concourse import bass2jax, mybir

    nc = build_program(N_NODES)
    bass2jax.install_neuronx_cc_hook()
    partition_name = (nc.partition_id_tensor.name
                      if nc.partition_id_tensor else None)
    in_names, out_names, out_avals, zero_outs = [], [], [], []
    for alloc in nc.m.functions[0].allocations:
        if not isinstance(alloc, mybir.MemoryLocationSet):
            continue
        name = alloc.memorylocations[0].name
        if alloc.kind == "ExternalInput":
            if name != partition_name:
                in_names.append(name)
        elif alloc.kind == "ExternalOutput":
            out_names.append(name)
            shape = tuple(alloc.tensor_shape)
            dtype = mybir.dt.np(alloc.dtype)
            out_avals.append(jax.core.ShapedArray(shape, dtype))
            zero_outs.append(np.zeros(shape, dtype))
    all_in = list(in_names) + list(out_names)
    if partition_name is not None:
        all_in.append(partition_name)

    def _body(*args):
        operands = list(args)
        if partition_name is not None:
            operands.append(bass2jax.partition_id_tensor())
        return tuple(bass2jax._bass_exec_p.bind(
            *operands, out_avals=tuple(out_avals), in_names=tuple(all_in),
            out_names=tuple(out_names), lowering_input_output_aliases=(),
            sim_require_finite=True, sim_require_nnan=True, nc=nc))

    devices = jax.devices()[:N_CORES]
    assert len(devices) == N_CORES, (
        f"need {N_CORES} neuron devices, found {len(jax.devices())}")
    mesh = Mesh(np.asarray(devices), ("core",))
    nio = len(in_names) + len(out_names)
    sharded = jax.jit(
        shard_map(_body, mesh=mesh,
                  in_specs=(PartitionSpec("core"),) * nio,
                  out_specs=(PartitionSpec("core"),) * len(out_names),
                  check_rep=False),
        keep_unused=True)
    sh = NamedSharding(mesh, PartitionSpec("core"))
    runner = dict(run=sharded, in_names=in_names, out_names=out_names,
                  zero_outs=zero_outs, sh=sh, jax=jax)
    _CACHE["runner"] = runner
    return runner


def kernel(inputs, ix_w, ix_b, ih_w, ih_b, ux_w, ux_b, uh_w, uh_b,
           fi_w, fi_b, fh_w, fh_b):
    """ChildSum TreeLSTM over a complete 4-ary tree of 65536 nodes on 8
    NeuronCores (SPMD, one AllGather at the level-4 frontier).
    Returns (h[0], c[0]) as float32 arrays of shape (256,)."""
    assert np.asarray(inputs).shape == (N_NODES, D)
    in_maps = prep_inputs(
        np.asarray(inputs, np.float32),
        np.asarray(ix_w, np.float32), np.asarray(ix_b, np.float32),
        np.asarray(ih_w, np.float32), np.asarray(ih_b, np.float32),
        np.asarray(ux_w, np.float32), np.asarray(ux_b, np.float32),
        np.asarray(uh_w, np.float32), np.asarray(uh_b, np.float32),
        np.asarray(fi_w, np.float32), np.asarray(fi_b, np.float32),
        np.asarray(fh_w, np.float32), np.asarray(fh_b, np.float32))
    r = _get_runner()
    jax = r["jax"]
    concat = [np.concatenate([in_maps[c][nm] for c in range(N_CORES)], axis=0)
              for nm in r["in_names"]]
    dev_in = [jax.device_put(a, r["sh"]) for a in concat]
    dev_zero = [jax.device_put(
        np.zeros((N_CORES * z.shape[0], *z.shape[1:]), z.dtype), r["sh"])
        for z in r["zero_outs"]]
    outs = r["run"](*dev_in, *dev_zero)
    res = {nm: np.asarray(outs[i]).reshape(N_CORES, P, 2)[0]
           for i, nm in enumerate(r["out_names"])}
    h0 = res["h0"].T.reshape(2 * P).astype(np.float32)
    c0 = res["c0"].T.reshape(2 * P).astype(np.float32)
    return h0, c0
